# revision 1
# baseline (speedup 1.0000x reference)
"""Trainium2 Bass kernel for nn_ButterflyLayer.

Reference computation:
    h   = x @ w_in.T                       [B, 2048]
    h   = butterfly(h, a_pad, b_pad)       11 stages of paired rotations
    out = h @ w_out.T + b_out              [B, 2048]

Key algebraic facts used here:
  * The butterfly is a linear map B on the 2048-dim:  out = x @ (w_out @ B @ w_in).T + b.
  * B factors as (M (x) I_128) @ blockdiag(D_0..D_15) where
      - D_c (128x128) is the composition of stages 0..6 restricted to 128-chunk c
        (those stages never mix across 128-aligned chunks), and
      - stages 7..10 use one scalar coefficient per 128-chunk, so they act as a
        16x16 matrix M on chunk indices, identically for every position inside
        a chunk.
  * So W_eff = (w_out @ (M (x) I)) @ blockdiag(D) @ w_in, and the butterfly
    costs only a block-sparse (128-wide) contraction instead of a dense one.

Host prep is limited to O(dim^2) parameter/layout work: expanding the tiny
rotation params (a_pad/b_pad) into the D_c blocks, folding the 16x16 chunk mix
M into w_out, and permuting operands into PE-friendly tiled layouts so every
device load is one large fully-contiguous DMA. All O(batch*dim^2) compute runs
on the NeuronCores.

All device data is bf16 (fp32 PSUM accumulation): rel-err ~4e-3 against the
fp32 reference (gate 2e-2), and it halves both HBM traffic and SBUF footprint
vs fp32. Same-session A/B on HW measured the bf16 kernel ~1.8x faster than
the fp32r version of the same algorithm.

Device program (SPMD on 8 cores, GB=1 batch-group x GN=8 column-groups —
every core builds a distinct 1/8 column slice of W_eff exactly once, no
duplicated build work device-wide, and streams the full batch):
  build:  g1[c]   = D_c^T-transform of the core's w_out' column slice
          W_eff^T[m, :] = sum_ft g1[ft][:, m-chunk].T @ w_in[ft-rows, :]
                  (bld2: stationary g1 m-chunks each feed 4 N=512 matmuls —
                   a fresh stationary per N=256 matmul measured ~2x slower
                   on HW because LDWEIGHTS can't hide at N=256 — then the
                   [m, dt*128+j] result transposes back into weff via a
                   DRAM round-trip with 2-byte transposing DMAs)
  main:   outT    = W_effT^T @ xT (+ bias), streamed over batch in 512-col
                    slabs; x loads on the SP HWDGE queue, out-stores on the
                    ACT HWDGE queue, first x slab prefetched on the ACT
                    queue during the build (pre_x)
"""

import sys

if "/opt/trn_rl_repo" not in sys.path:
    sys.path.insert(0, "/opt/trn_rl_repo")

import numpy as np
import ml_dtypes

import concourse.bass as bass
import concourse.mybir as mybir
import concourse.tile as tile
from concourse import bacc
from concourse.bass import ts
from concourse.bass_utils import run_bass_kernel_spmd

DIM = 2048
LOG_DIM = 11
BATCH = 16384
N_CORES = 8
P = 128                # partitions
NB = 512               # main-loop moving free dim (one PSUM bank of fp32)
NCHUNK = DIM // P      # 16
F32 = mybir.dt.float32
BF16 = mybir.dt.bfloat16
NP_BF16 = ml_dtypes.bfloat16

# default sharding: GB batch-groups x GN column-groups (GB*GN == 8).
# GB=1/GN=8: every core builds a distinct 1/8 column-slice of W_eff exactly
# once (minimal PE work) and streams the full batch (DMA-heavier but still
# under the PE roofline).
GB = 1
GN = 8


def _cfg(gb, gn):
    msh = BATCH // gb          # batch rows per core
    nsl = DIM // gn            # output columns per core
    n_mb = msh // NB           # 512-wide batch blocks
    n_nt = nsl // P            # 128-wide out-column tiles
    bnb = min(NB, nsl)         # build free width
    n_bnb = nsl // bnb         # build column blocks
    return msh, nsl, n_mb, n_nt, bnb, n_bnb


# ---------------------------------------------------------------- host math

def _butterfly_dense(a_pad, b_pad, stages):
    """Dense matrix of the butterfly restricted to `stages` (float64).

    Returns Bm with butterfly(v) = Bm @ v for v in R^DIM.
    """
    x = np.eye(DIM, dtype=np.float64)  # rows: basis vectors
    for l in stages:
        bs = 1 << l
        nb = DIM // (2 * bs)
        a = a_pad[l, :nb].astype(np.float64)[None, :, None]
        b = b_pad[l, :nb].astype(np.float64)[None, :, None]
        xv = x.reshape(DIM, nb, 2, bs)
        x0 = xv[:, :, 0, :]
        x1 = xv[:, :, 1, :]
        top = a * x0 + b * x1
        bot = -b * x0 + a * x1
        x = np.stack([top, bot], axis=2).reshape(DIM, DIM)
    return x.T  # butterfly(I)[r] = Bm @ e_r, so butterfly(I) = Bm.T


def _host_prep(w_in, w_out, a_pad, b_pad):
    """Expand butterfly params; permute operands into tiled device layouts."""
    d_full = _butterfly_dense(a_pad, b_pad, range(7))           # blockdiag(D_c)
    m_full = _butterfly_dense(a_pad, b_pad, range(7, LOG_DIM))  # M (x) I_128
    m_small = np.ascontiguousarray(m_full[::P, ::P])            # [16, 16]

    # dstack[k, c*128+j] = D_c[k, j]  (one contiguous [128, 2048] tile row)
    d_arr = np.stack(
        [d_full[c * P:(c + 1) * P, c * P:(c + 1) * P] for c in range(NCHUNK)]
    )                                                           # [c, k, j]
    dstack = np.ascontiguousarray(
        d_arr.transpose(1, 0, 2).reshape(P, DIM)).astype(np.float32)

    # w_out' = w_out @ (M (x) I)
    w_out64 = w_out.astype(np.float64).reshape(DIM, NCHUNK, P)
    w_out_p = np.einsum("icj,cd->idj", w_out64, m_small).reshape(DIM, DIM)
    wopT = np.ascontiguousarray(w_out_p.T).astype(np.float32)   # [k, n]

    # w2[dt*128+p, ft*128+d] = w_in[ft*128+p, dt*128+d]
    w2 = np.ascontiguousarray(
        w_in.reshape(NCHUNK, P, NCHUNK, P).transpose(2, 1, 0, 3)
        .reshape(DIM, DIM))
    return dstack, wopT, w2


def _per_core_weights(b_out, wopT, g, gn, np_dt):
    _, nsl, _, n_nt, bnb, n_bnb = _cfg(1, gn)
    # g0t[nb*128+p, c*bnb+m] = wopT[c*128+p, g*nsl + nb*bnb+m]
    ws = wopT[:, g * nsl:(g + 1) * nsl]
    g0t = np.ascontiguousarray(
        ws.reshape(NCHUNK, P, n_bnb, bnb).transpose(2, 1, 0, 3)
        .reshape(n_bnb * P, NCHUNK * bnb)).astype(np_dt)
    # bias2[p, nt] = b_out[g*nsl + nt*128 + p]
    bias2 = np.ascontiguousarray(
        b_out[g * nsl:(g + 1) * nsl].reshape(n_nt, P).T).astype(np.float32)
    return g0t, bias2


def _per_group_x(x_cast, b, gb):
    msh, _, n_mb, _, _, _ = _cfg(gb, 1)
    # x2[mb*128+p, dt*512+m] = x[b*msh + mb*512+m, dt*128+p]
    xs = x_cast[b * msh:(b + 1) * msh, :]
    return np.ascontiguousarray(
        xs.reshape(n_mb, NB, NCHUNK, P).transpose(0, 3, 2, 1)
        .reshape(n_mb * P, NCHUNK * NB))


# ------------------------------------------------------------- device build

def build_nc(gb=GB, gn=GN, xs_bufs=3, wslab_bufs=6, warmup=8,
             build_reps=1, main_reps=1, io="bf16", xs_q="sync",
             st_q="scalar", pre_x=2, pre_x_q="scalar", g0_whole=True,
             bld2=True, mgrp=1):
    msh, nsl, n_mb, n_nt, bnb, n_bnb = _cfg(gb, gn)
    IDT = BF16 if io == "bf16" else mybir.dt.float32r
    ODT = BF16 if io == "bf16" else F32
    assert not (bld2 and io != "bf16"), "bld2 needs 2-byte transposing DMA"
    assert not (bld2 and n_bnb != 1), "bld2 assumes a single build col block"
    nc = bacc.Bacc("TRN2", target_bir_lowering=False, debug=False,
                   num_devices=N_CORES)

    xt = nc.dram_tensor("xt", [n_mb * P, NCHUNK * NB], IDT,
                        kind="ExternalInput")
    # bld2 consumes w_in row-chunks in natural layout; the classic build
    # consumes the (dt,ft)-permuted w2 layout
    w2 = nc.dram_tensor("w2", [DIM, DIM], IDT, kind="ExternalInput")
    g0t = nc.dram_tensor("g0t", [n_bnb * P, NCHUNK * bnb], IDT,
                         kind="ExternalInput")
    dstk = nc.dram_tensor("dstk", [P, DIM], IDT, kind="ExternalInput")
    bias = nc.dram_tensor("bias", [P, n_nt], F32, kind="ExternalInput")
    # outt[(mb*n_nt+nt)*128+p, m] = out[b*msh+mb*512+m, g*nsl+nt*128+p]
    outt = nc.dram_tensor("outt", [n_mb * n_nt * P, NB], ODT,
                          kind="ExternalOutput")

    with tile.TileContext(nc) as tc:
        with tc.tile_pool(name="geom", bufs=1) as geom:          # persistent
            # --- persistent tiles
            dblk_sb = geom.tile([P, DIM], IDT)
            nc.sync.dma_start(out=dblk_sb, in_=dstk[:, :])
            bias_sb = geom.tile([P, n_nt], F32)
            nc.sync.dma_start(out=bias_sb, in_=bias[:, :])
            weff_sb = [geom.tile([P, nsl], IDT, name=f"weff{dt}")
                       for dt in range(NCHUNK)]
            # PE warmup while the first DMAs land (HAM ramp)
            if warmup:
                wup = geom.tile([P, NB], BF16, name="wup")
                nc.vector.memset(wup, 0.0)
                with tc.tile_pool(name="psw", bufs=2, space="PSUM") as psw:
                    for _ in range(warmup):
                        ptw = psw.tile([P, NB], F32, tag="ps")
                        nc.tensor.matmul(ptw[:, :], wup[:, :P], wup,
                                         start=True, stop=True)

            # optional early prefetch of the first x slabs on a spare queue
            # (rep 0 of the main loop consumes these without re-loading)
            pre_tiles = []
            for i in range(pre_x):
                xsp = geom.tile([P, NCHUNK * NB], IDT, name=f"xs_pre{i}")
                getattr(nc, pre_x_q).dma_start(
                    out=xsp, in_=xt[i * P:(i + 1) * P, :])
                pre_tiles.append(xsp)

            # --- build W_effT = (w_in.T @ blockdiag(D).T @ w_out'.T)[:, n-slice]
            for _rep in range(build_reps):
                with tc.tile_pool(name="bld", bufs=2) as bld, \
                     tc.tile_pool(name="g1p", bufs=1) as g1p, \
                     tc.tile_pool(name="psb", bufs=8, space="PSUM") as psb:
                    g1_sb = [g1p.tile([P, nsl], IDT, name=f"g1_{c}")
                             for c in range(NCHUNK)]
                    # g1[c] = D_c^T-transform of w_out'^T chunk c
                    for nb in range(n_bnb):
                        if g0_whole:
                            g0row = bld.tile([P, NCHUNK * bnb], IDT,
                                             tag="g0w", bufs=2)
                            nc.sync.dma_start(
                                out=g0row, in_=g0t[nb * P:(nb + 1) * P, :])
                        for c in range(NCHUNK):
                            if g0_whole:
                                g0c = g0row[:, ts(c, bnb)]
                            else:
                                g0c = bld.tile([P, bnb], IDT, tag="g0",
                                               bufs=4)
                                nc.sync.dma_start(
                                    out=g0c,
                                    in_=g0t[nb * P:(nb + 1) * P, ts(c, bnb)])
                            pt = psb.tile([P, bnb], F32, tag="ps")
                            nc.tensor.matmul(pt[:, :], dblk_sb[:, ts(c, P)],
                                             g0c, start=True, stop=True)
                            nc.any.tensor_copy(g1_sb[c][:, ts(nb, bnb)],
                                               pt[:, :])
                    if bld2:
                        # W_eff[m, J] = sum_ft g1[ft][:,m].T @ w_in[ft-rows, J]
                        # — stationary g1 m-chunks feed 4 N=512 matmuls each
                        # (vs a fresh stationary per N=256 matmul), then the
                        # [m, J] result is transposed back into weff_sb[dt]
                        # via a DRAM round-trip with transposing DMAs.
                        n_mc = nsl // P
                        n_q = DIM // NB
                        with tc.tile_pool(name="wt", bufs=1,
                                          space="DRAM") as wtp:
                            wtmp = wtp.tile([nsl, DIM], IDT)
                            pts = [[psb.tile([P, NB], F32, tag="ps",
                                             name=f"pt{_mc}_{_q}")
                                    for _q in range(n_q)]
                                   for _mc in range(n_mc)]
                            for ft in range(NCHUNK):
                                wr = bld.tile([P, DIM], IDT, tag="wslab",
                                              bufs=wslab_bufs)
                                nc.sync.dma_start(
                                    out=wr, in_=w2[ft * P:(ft + 1) * P, :])
                                for mc in range(n_mc):
                                    for q in range(n_q):
                                        nc.tensor.matmul(
                                            pts[mc][q][:, :],
                                            g1_sb[ft][:, ts(mc, P)],
                                            wr[:, ts(q, NB)],
                                            start=(ft == 0),
                                            stop=(ft == NCHUNK - 1))
                            for mc in range(n_mc):
                                stage = bld.tile([P, DIM], IDT, tag="stg",
                                                 bufs=2)
                                for q in range(n_q):
                                    nc.any.tensor_copy(stage[:, ts(q, NB)],
                                                       pts[mc][q][:, :])
                                nc.sync.dma_start(
                                    out=wtmp[mc * P:(mc + 1) * P, :],
                                    in_=stage)
                            # transposes stay on the SAME queue as the
                            # wtmp stage stores: queue FIFO is what orders
                            # the DRAM read-after-write (cross-queue DRAM
                            # deps are NOT tracked — splitting the chain
                            # across queues raced and corrupted weff; and
                            # per-[128,128]-chunk transposes fall below the
                            # xbar tile size and degrade to slow AP-swap
                            # descriptors, +32 us in sim)
                            for dt in range(NCHUNK):
                                nc.sync.dma_start_transpose(
                                    out=weff_sb[dt][:, :],
                                    in_=wtmp[:, ts(dt, P)])
                    else:
                        # W_effT[dt] = sum_ft w_in[ft, dt].T @ g1[ft]
                        for dt in range(NCHUNK):
                            wslab = bld.tile([P, DIM], IDT, tag="wslab",
                                             bufs=wslab_bufs)
                            nc.sync.dma_start(out=wslab,
                                              in_=w2[dt * P:(dt + 1) * P, :])
                            for nb in range(n_bnb):
                                pt = psb.tile([P, bnb], F32, tag="ps")
                                for ft in range(NCHUNK):
                                    nc.tensor.matmul(
                                        pt[:, :], wslab[:, ts(ft, P)],
                                        g1_sb[ft][:, ts(nb, bnb)],
                                        start=(ft == 0),
                                        stop=(ft == NCHUNK - 1))
                                nc.any.tensor_copy(weff_sb[dt][:, ts(nb, bnb)],
                                                   pt[:, :])

            # --- main: outT[nt, mb] = sum_dt W_effT[dt, nt].T @ xT[dt, mb] + bias
            # (pools opened here, not alongside the build: early x-prefetch
            # DMAs jumping the queue ahead of build loads measured SLOWER on
            # HW despite modeling better)
            xs_eng = getattr(nc, xs_q)
            st_engs = ([nc.scalar, nc.gpsimd] if st_q == "alt"
                       else [getattr(nc, st_q)])
            assert n_mb % mgrp == 0
            for _rep in range(main_reps):
                with tc.tile_pool(name="mn", bufs=xs_bufs) as mn, \
                     tc.tile_pool(name="ob", bufs=4) as ob, \
                     tc.tile_pool(name="psm", bufs=8, space="PSUM") as psm:
                    for mbg in range(n_mb // mgrp):
                        xs_list = []
                        for i in range(mgrp):
                            mb = mbg * mgrp + i
                            if _rep == 0 and mb < pre_x:
                                xs = pre_tiles[mb]
                            else:
                                xs = mn.tile([P, NCHUNK * NB], IDT, tag="xs")
                                xs_eng.dma_start(
                                    out=xs, in_=xt[mb * P:(mb + 1) * P, :])
                            xs_list.append(xs)
                        for nt in range(n_nt):
                            pts = [psm.tile([P, NB], F32, tag="ps",
                                             name=f"pt{_i}")
                                   for _i in range(mgrp)]
                            # mb-inner so the stationary weff chunk is
                            # reused across the mgrp batch blocks
                            for dt in range(NCHUNK):
                                for i in range(mgrp):
                                    nc.tensor.matmul(
                                        pts[i][:, :],
                                        weff_sb[dt][:, ts(nt, P)],
                                        xs_list[i][:, ts(dt, NB)],
                                        start=(dt == 0),
                                        stop=(dt == NCHUNK - 1))
                            for i in range(mgrp):
                                mb = mbg * mgrp + i
                                osb = ob.tile([P, NB], ODT, tag="osb")
                                nc.scalar.activation(
                                    osb, pts[i][:, :],
                                    mybir.ActivationFunctionType.Identity,
                                    bias=bias_sb[:, nt:nt + 1])
                                # out-stores on a third queue, away from both
                                # the build loads (SP) and the x loads
                                idx = mb * n_nt + nt
                                st_engs[idx % len(st_engs)].dma_start(
                                    out=outt[idx * P:(idx + 1) * P, :],
                                    in_=osb)

    nc.compile()
    return nc


_NC_CACHE = {}


def _get_nc(gb, gn):
    key = (gb, gn)
    if key not in _NC_CACHE:
        _NC_CACHE[key] = build_nc(gb, gn)
    return _NC_CACHE[key]


# ------------------------------------------------------------------ driver

def _make_in_maps(x, w_in, w_out, b_out, a_pad, b_pad, gb=GB, gn=GN,
                  io="bf16", bld2=True):
    np_dt = NP_BF16 if io == "bf16" else np.float32
    dstack, wopT, w2 = _host_prep(w_in, w_out, a_pad, b_pad)
    dstack = dstack.astype(np_dt)
    # bld2 streams w_in row-chunks in natural layout instead of the permuted
    # w2 (same dram tensor name/shape, different host-side contents)
    w2 = (w_in if bld2 else w2).astype(np_dt)
    x_cast = x.astype(np_dt)
    x2_by_b = {b: _per_group_x(x_cast, b, gb) for b in range(gb)}
    wt_by_g = {g: _per_core_weights(b_out, wopT, g, gn, np_dt)
               for g in range(gn)}
    in_maps = []
    for core in range(N_CORES):
        b, g = divmod(core, gn)
        g0t, bias2 = wt_by_g[g]
        in_maps.append({
            "xt": x2_by_b[b],
            "w2": w2,
            "g0t": g0t,
            "dstk": dstack,
            "bias": bias2,
        })
    return in_maps


def _assemble(results, gb=GB, gn=GN):
    msh, nsl, n_mb, n_nt, _, _ = _cfg(gb, gn)
    out = np.empty((BATCH, DIM), dtype=np.float32)
    for core in range(N_CORES):
        b, g = divmod(core, gn)
        # outt rows [(mb*n_nt+nt)*128+p], cols m
        arr = results[core]["outt"].astype(np.float32)
        arr = arr.reshape(n_mb, n_nt, P, NB)
        # -> [mb, m, nt, p]
        out[b * msh:(b + 1) * msh, g * nsl:(g + 1) * nsl] = \
            arr.transpose(0, 3, 1, 2).reshape(msh, nsl)
    return out


def kernel(x, w_in, w_out, b_out, a_pad, b_pad):
    x = np.asarray(x, dtype=np.float32)
    w_in = np.asarray(w_in, dtype=np.float32)
    w_out = np.asarray(w_out, dtype=np.float32)
    b_out = np.asarray(b_out, dtype=np.float32)
    a_pad = np.asarray(a_pad, dtype=np.float32)
    b_pad = np.asarray(b_pad, dtype=np.float32)
    nc = _get_nc(GB, GN)
    in_maps = _make_in_maps(x, w_in, w_out, b_out, a_pad, b_pad, GB, GN)
    res = run_bass_kernel_spmd(nc, in_maps, core_ids=list(range(N_CORES)))
    return _assemble(res.results, GB, GN)



# revision 29
# speedup vs baseline: 4.9728x; 4.9728x over previous
"""Trainium2 Bass kernel for nn_ButterflyLayer.

Reference computation:
    h   = x @ w_in.T                       [B, 2048]
    h   = butterfly(h, a_pad, b_pad)       11 stages of paired rotations
    out = h @ w_out.T + b_out              [B, 2048]

Key algebraic facts used here:
  * The butterfly is a linear map B on the 2048-dim:  out = x @ (w_out @ B @ w_in).T + b.
  * B factors as (M (x) I_128) @ blockdiag(D_0..D_15) where
      - D_c (128x128) is the composition of stages 0..6 restricted to 128-chunk c
        (those stages never mix across 128-aligned chunks), and
      - stages 7..10 use one scalar coefficient per 128-chunk, so they act as a
        16x16 matrix M on chunk indices, identically for every position inside
        a chunk.
  * So W_eff = (w_out @ (M (x) I)) @ blockdiag(D) @ w_in, and the butterfly
    costs only a block-sparse (128-wide) contraction instead of a dense one.

Host prep is limited to O(dim^2) parameter/layout work: expanding the tiny
rotation params (a_pad/b_pad) into the D_c blocks, folding the 16x16 chunk mix
M into w_out, and permuting operands into PE-friendly tiled layouts so every
device load is one large fully-contiguous DMA. All O(batch*dim^2) compute runs
on the NeuronCores.

All device data is bf16 (fp32 PSUM accumulation): rel-err ~4e-3 against the
fp32 reference (gate 2e-2), and it halves both HBM traffic and SBUF footprint
vs fp32. Same-session A/B on HW measured the bf16 kernel ~1.8x faster than
the fp32r version of the same algorithm.

Device program (SPMD on 8 cores, GB=1 batch-group x GN=8 column-groups —
every core builds a distinct 1/8 column slice of W_eff exactly once, no
duplicated build work device-wide, and streams the full batch):
  build:  g1[c]   = D_c^T-transform of the core's w_out' column slice
          W_eff^T[m, :] = sum_ft g1[ft][:, m-chunk].T @ w_in[ft-rows, :]
                  (bld2: stationary g1 m-chunks each feed 4 N=512 matmuls —
                   a fresh stationary per N=256 matmul measured ~2x slower
                   on HW because LDWEIGHTS can't hide at N=256 — then the
                   [m, J] result transposes into weff on the PE via identity
                   matmuls, tr="pe": two mc passes of 4 PSUM banks + 2
                   transpose banks. This replaced a DRAM round-trip with
                   transposing DMAs that gated the build->main handoff:
                   full-body slope 330 -> 305 us, build-only marginal
                   47.5 -> 18.2 us, same-session A/B 2026-08-10)
  main:   outT    = W_effT^T @ xT (+ bias), streamed over batch in 512-col
                    slabs; x loads on the SP HWDGE queue, out-stores on the
                    ACT HWDGE queue, first x slab prefetched on the ACT
                    queue during the build (pre_x)

Measured dead ends (same-session interleaved A/B, 2026-08-10 — see also
the session memory): mgrp stationary-reuse 2/4/8 (LDWEIGHTS already hides
at N=512; nox-isolated mgrp=4 == mgrp=1), wide 2KB-line out-stores (wst),
x loads alternated across both HWDGE rings, 2-slab packed x tiles (xg=2),
w_in build loads on the ACT ring, gb/gn=2x4 (348 vs 306 us). Main loop
runs ~236 us PE-pure vs 221 us gap-table floor; the residual is NX issue +
HAM micro-idle overhead, not LDWEIGHTS or DMA.
"""

import sys

if "/opt/trn_rl_repo" not in sys.path:
    sys.path.insert(0, "/opt/trn_rl_repo")

import numpy as np
import ml_dtypes

import concourse.bass as bass
import concourse.mybir as mybir
import concourse.tile as tile
from concourse import bacc
from concourse.bass import ts
from concourse.bass_utils import run_bass_kernel_spmd

DIM = 2048
LOG_DIM = 11
BATCH = 16384
N_CORES = 8
P = 128                # partitions
NB = 512               # main-loop moving free dim (one PSUM bank of fp32)
NCHUNK = DIM // P      # 16
F32 = mybir.dt.float32
BF16 = mybir.dt.bfloat16
NP_BF16 = ml_dtypes.bfloat16

# default sharding: GB batch-groups x GN column-groups (GB*GN == 8).
# GB=1/GN=8: every core builds a distinct 1/8 column-slice of W_eff exactly
# once (minimal PE work) and streams the full batch (DMA-heavier but still
# under the PE roofline). 2x4 re-measured 2026-08-10: full-body slope 348 vs
# 306 us — the doubled build cost dominates the halved x traffic; dead.
GB = 1
GN = 8
# packed-x group size (xg): 2 measured slightly SLOWER (312 vs 307) — keep 1
XG = 1


def _cfg(gb, gn):
    msh = BATCH // gb          # batch rows per core
    nsl = DIM // gn            # output columns per core
    n_mb = msh // NB           # 512-wide batch blocks
    n_nt = nsl // P            # 128-wide out-column tiles
    bnb = min(NB, nsl)         # build free width
    n_bnb = nsl // bnb         # build column blocks
    return msh, nsl, n_mb, n_nt, bnb, n_bnb


# ---------------------------------------------------------------- host math

def _butterfly_dense(a_pad, b_pad, stages):
    """Dense matrix of the butterfly restricted to `stages` (float64).

    Returns Bm with butterfly(v) = Bm @ v for v in R^DIM.
    """
    x = np.eye(DIM, dtype=np.float64)  # rows: basis vectors
    for l in stages:
        bs = 1 << l
        nb = DIM // (2 * bs)
        a = a_pad[l, :nb].astype(np.float64)[None, :, None]
        b = b_pad[l, :nb].astype(np.float64)[None, :, None]
        xv = x.reshape(DIM, nb, 2, bs)
        x0 = xv[:, :, 0, :]
        x1 = xv[:, :, 1, :]
        top = a * x0 + b * x1
        bot = -b * x0 + a * x1
        x = np.stack([top, bot], axis=2).reshape(DIM, DIM)
    return x.T  # butterfly(I)[r] = Bm @ e_r, so butterfly(I) = Bm.T


def _host_prep(w_in, w_out, a_pad, b_pad):
    """Expand butterfly params; permute operands into tiled device layouts."""
    d_full = _butterfly_dense(a_pad, b_pad, range(7))           # blockdiag(D_c)
    m_full = _butterfly_dense(a_pad, b_pad, range(7, LOG_DIM))  # M (x) I_128
    m_small = np.ascontiguousarray(m_full[::P, ::P])            # [16, 16]

    # dstack[k, c*128+j] = D_c[k, j]  (one contiguous [128, 2048] tile row)
    d_arr = np.stack(
        [d_full[c * P:(c + 1) * P, c * P:(c + 1) * P] for c in range(NCHUNK)]
    )                                                           # [c, k, j]
    dstack = np.ascontiguousarray(
        d_arr.transpose(1, 0, 2).reshape(P, DIM)).astype(np.float32)

    # w_out' = w_out @ (M (x) I)
    w_out64 = w_out.astype(np.float64).reshape(DIM, NCHUNK, P)
    w_out_p = np.einsum("icj,cd->idj", w_out64, m_small).reshape(DIM, DIM)
    wopT = np.ascontiguousarray(w_out_p.T).astype(np.float32)   # [k, n]

    # w2[dt*128+p, ft*128+d] = w_in[ft*128+p, dt*128+d]
    w2 = np.ascontiguousarray(
        w_in.reshape(NCHUNK, P, NCHUNK, P).transpose(2, 1, 0, 3)
        .reshape(DIM, DIM))
    return dstack, wopT, w2


def _per_core_weights(b_out, wopT, g, gn, np_dt):
    _, nsl, _, n_nt, bnb, n_bnb = _cfg(1, gn)
    # g0t[nb*128+p, c*bnb+m] = wopT[c*128+p, g*nsl + nb*bnb+m]
    ws = wopT[:, g * nsl:(g + 1) * nsl]
    g0t = np.ascontiguousarray(
        ws.reshape(NCHUNK, P, n_bnb, bnb).transpose(2, 1, 0, 3)
        .reshape(n_bnb * P, NCHUNK * bnb)).astype(np_dt)
    # bias2[p, nt] = b_out[g*nsl + nt*128 + p]
    bias2 = np.ascontiguousarray(
        b_out[g * nsl:(g + 1) * nsl].reshape(n_nt, P).T).astype(np.float32)
    return g0t, bias2


def _per_group_x(x_cast, b, gb, xg=1):
    msh, _, n_mb, _, _, _ = _cfg(gb, 1)
    # x2[mb*128+p, dt*512+m] = x[b*msh + mb*512+m, dt*128+p]
    # xg>1: x2[g*128+p, (s*16+dt)*512+m] = x[b*msh + (g*xg+s)*512+m, dt*128+p]
    xs = x_cast[b * msh:(b + 1) * msh, :]
    n_grp = n_mb // xg
    return np.ascontiguousarray(
        xs.reshape(n_grp, xg, NB, NCHUNK, P).transpose(0, 4, 1, 3, 2)
        .reshape(n_grp * P, xg * NCHUNK * NB))


# ------------------------------------------------------------- device build

def build_nc(gb=GB, gn=GN, xs_bufs=3, wslab_bufs=6, warmup=8,
             build_reps=1, main_reps=1, io="bf16", xs_q="sync",
             st_q="scalar", pre_x=2, pre_x_q="scalar", g0_whole=True,
             bld2=True, mgrp=1, tr="pe", nox=False, nostore=False,
             wst=False, wsl_q="sync", xg=XG):
    msh, nsl, n_mb, n_nt, bnb, n_bnb = _cfg(gb, gn)
    IDT = BF16 if io == "bf16" else mybir.dt.float32r
    ODT = BF16 if io == "bf16" else F32
    assert not (bld2 and io != "bf16"), "bld2 needs 2-byte transposing DMA"
    assert not (bld2 and n_bnb != 1), "bld2 assumes a single build col block"
    nc = bacc.Bacc("TRN2", target_bir_lowering=False, debug=False,
                   num_devices=N_CORES)

    # xg>1 packs xg consecutive batch slabs into one DRAM row-block /
    # SBUF tile, halving the x-load DMA count and doubling the contiguous
    # line length (16KB -> 32KB per partition row)
    assert n_mb % xg == 0 and not (xg > 1 and mgrp > 1)
    xt = nc.dram_tensor("xt", [(n_mb // xg) * P, xg * NCHUNK * NB], IDT,
                        kind="ExternalInput")
    # bld2 consumes w_in row-chunks in natural layout; the classic build
    # consumes the (dt,ft)-permuted w2 layout
    w2 = nc.dram_tensor("w2", [DIM, DIM], IDT, kind="ExternalInput")
    g0t = nc.dram_tensor("g0t", [n_bnb * P, NCHUNK * bnb], IDT,
                         kind="ExternalInput")
    dstk = nc.dram_tensor("dstk", [P, DIM], IDT, kind="ExternalInput")
    bias = nc.dram_tensor("bias", [P, n_nt], F32, kind="ExternalInput")
    # outt[(mb*n_nt+nt)*128+p, m] = out[b*msh+mb*512+m, g*nsl+nt*128+p]
    # wst: outt[mb*128+p, nt*512+m] — both nt tiles of a batch slab merge
    # into one store with 2KB contiguous DRAM lines (vs 1KB), which the
    # SDMA engines move more efficiently. _assemble sniffs the layout from
    # the result shape.
    if wst:
        outt = nc.dram_tensor("outt", [n_mb * P, n_nt * NB], ODT,
                              kind="ExternalOutput")
    else:
        outt = nc.dram_tensor("outt", [n_mb * n_nt * P, NB], ODT,
                              kind="ExternalOutput")

    with tile.TileContext(nc) as tc:
        with tc.tile_pool(name="geom", bufs=1) as geom:          # persistent
            # --- persistent tiles
            dblk_sb = geom.tile([P, DIM], IDT)
            nc.sync.dma_start(out=dblk_sb, in_=dstk[:, :])
            bias_sb = geom.tile([P, n_nt], F32)
            nc.sync.dma_start(out=bias_sb, in_=bias[:, :])
            weff_sb = [geom.tile([P, nsl], IDT, name=f"weff{dt}")
                       for dt in range(NCHUNK)]
            if tr == "pe":
                from concourse.masks import make_identity
                ident = geom.tile([P, P], IDT, name="ident")
                make_identity(nc, ident)
            # PE warmup while the first DMAs land (HAM ramp)
            if warmup:
                wup = geom.tile([P, NB], BF16, name="wup")
                nc.vector.memset(wup, 0.0)
                with tc.tile_pool(name="psw", bufs=2, space="PSUM") as psw:
                    for _ in range(warmup):
                        ptw = psw.tile([P, NB], F32, tag="ps")
                        nc.tensor.matmul(ptw[:, :], wup[:, :P], wup,
                                         start=True, stop=True)

            # optional early prefetch of the first x slabs on a spare queue
            # (rep 0 of the main loop consumes these without re-loading)
            pre_tiles = []
            for i in range(pre_x):
                xsp = geom.tile([P, xg * NCHUNK * NB], IDT,
                                name=f"xs_pre{i}")
                getattr(nc, pre_x_q).dma_start(
                    out=xsp, in_=xt[i * P:(i + 1) * P, :])
                pre_tiles.append(xsp)

            # --- build W_effT = (w_in.T @ blockdiag(D).T @ w_out'.T)[:, n-slice]
            for _rep in range(build_reps):
                with tc.tile_pool(name="bld", bufs=2) as bld, \
                     tc.tile_pool(name="g1p", bufs=1) as g1p, \
                     tc.tile_pool(name="psb", bufs=8, space="PSUM") as psb:
                    g1_sb = [g1p.tile([P, nsl], IDT, name=f"g1_{c}")
                             for c in range(NCHUNK)]
                    # g1[c] = D_c^T-transform of w_out'^T chunk c
                    for nb in range(n_bnb):
                        if g0_whole:
                            g0row = bld.tile([P, NCHUNK * bnb], IDT,
                                             tag="g0w", bufs=2)
                            nc.sync.dma_start(
                                out=g0row, in_=g0t[nb * P:(nb + 1) * P, :])
                        for c in range(NCHUNK):
                            if g0_whole:
                                g0c = g0row[:, ts(c, bnb)]
                            else:
                                g0c = bld.tile([P, bnb], IDT, tag="g0",
                                               bufs=4)
                                nc.sync.dma_start(
                                    out=g0c,
                                    in_=g0t[nb * P:(nb + 1) * P, ts(c, bnb)])
                            pt = psb.tile([P, bnb], F32, tag="ps",
                                          bufs=(2 if tr == "pe" else None))
                            nc.tensor.matmul(pt[:, :], dblk_sb[:, ts(c, P)],
                                             g0c, start=True, stop=True)
                            nc.any.tensor_copy(g1_sb[c][:, ts(nb, bnb)],
                                               pt[:, :])
                    if bld2 and tr == "pe":
                        # Same math as the tr="dma" branch below, but the
                        # [m, J] -> [J, m] transpose runs on the PE (identity
                        # matmul) instead of a DRAM round-trip, removing the
                        # store+transposing-load chain that gates the
                        # build->main handoff. Two mc passes of 4 PSUM banks
                        # each leave PSUM room for the transpose outputs;
                        # w_in row slabs are pinned in SBUF across both
                        # passes so they stream only once.
                        n_mc = nsl // P
                        n_q = DIM // NB
                        wslabs = []
                        for ft in range(NCHUNK):
                            wr = bld.tile([P, DIM], IDT, tag=f"ws{ft}",
                                          bufs=1)
                            # wsl_q="scalar" puts these on the ACT ring so
                            # rep r+1's 8 MB of w_in prefetches during rep
                            # r's main loop instead of queueing behind its
                            # x loads on the SP ring
                            getattr(nc, wsl_q).dma_start(
                                out=wr, in_=w2[ft * P:(ft + 1) * P, :])
                            wslabs.append(wr)
                        for mc in range(n_mc):
                            pts = [psb.tile([P, NB], F32, tag="mm",
                                            name=f"pt{mc}_{q}", bufs=4)
                                   for q in range(n_q)]
                            for ft in range(NCHUNK):
                                for q in range(n_q):
                                    nc.tensor.matmul(
                                        pts[q][:, :],
                                        g1_sb[ft][:, ts(mc, P)],
                                        wslabs[ft][:, ts(q, NB)],
                                        start=(ft == 0),
                                        stop=(ft == NCHUNK - 1))
                            stage = bld.tile([P, DIM], IDT, tag="stg",
                                             bufs=2)
                            for q in range(n_q):
                                nc.any.tensor_copy(stage[:, ts(q, NB)],
                                                   pts[q][:, :])
                                for dt in range(q * (NB // P),
                                                (q + 1) * (NB // P)):
                                    ptr = psb.tile([P, P], BF16, tag="tr",
                                                   bufs=2)
                                    nc.tensor.transpose(
                                        ptr[:, :], stage[:, ts(dt, P)],
                                        ident)
                                    nc.any.tensor_copy(
                                        weff_sb[dt][:, ts(mc, P)],
                                        ptr[:, :])
                    elif bld2:
                        # W_eff[m, J] = sum_ft g1[ft][:,m].T @ w_in[ft-rows, J]
                        # — stationary g1 m-chunks feed 4 N=512 matmuls each
                        # (vs a fresh stationary per N=256 matmul), then the
                        # [m, J] result is transposed back into weff_sb[dt]
                        # via a DRAM round-trip with transposing DMAs.
                        n_mc = nsl // P
                        n_q = DIM // NB
                        with tc.tile_pool(name="wt", bufs=1,
                                          space="DRAM") as wtp:
                            wtmp = wtp.tile([nsl, DIM], IDT)
                            pts = [[psb.tile([P, NB], F32, tag="ps",
                                             name=f"pt{_mc}_{_q}")
                                    for _q in range(n_q)]
                                   for _mc in range(n_mc)]
                            for ft in range(NCHUNK):
                                wr = bld.tile([P, DIM], IDT, tag="wslab",
                                              bufs=wslab_bufs)
                                nc.sync.dma_start(
                                    out=wr, in_=w2[ft * P:(ft + 1) * P, :])
                                for mc in range(n_mc):
                                    for q in range(n_q):
                                        nc.tensor.matmul(
                                            pts[mc][q][:, :],
                                            g1_sb[ft][:, ts(mc, P)],
                                            wr[:, ts(q, NB)],
                                            start=(ft == 0),
                                            stop=(ft == NCHUNK - 1))
                            for mc in range(n_mc):
                                stage = bld.tile([P, DIM], IDT, tag="stg",
                                                 bufs=2)
                                for q in range(n_q):
                                    nc.any.tensor_copy(stage[:, ts(q, NB)],
                                                       pts[mc][q][:, :])
                                nc.sync.dma_start(
                                    out=wtmp[mc * P:(mc + 1) * P, :],
                                    in_=stage)
                            # transposes stay on the SAME queue as the
                            # wtmp stage stores: queue FIFO is what orders
                            # the DRAM read-after-write (cross-queue DRAM
                            # deps are NOT tracked — splitting the chain
                            # across queues raced and corrupted weff; and
                            # per-[128,128]-chunk transposes fall below the
                            # xbar tile size and degrade to slow AP-swap
                            # descriptors, +32 us in sim)
                            for dt in range(NCHUNK):
                                nc.sync.dma_start_transpose(
                                    out=weff_sb[dt][:, :],
                                    in_=wtmp[:, ts(dt, P)])
                    else:
                        # W_effT[dt] = sum_ft w_in[ft, dt].T @ g1[ft]
                        for dt in range(NCHUNK):
                            wslab = bld.tile([P, DIM], IDT, tag="wslab",
                                             bufs=wslab_bufs)
                            nc.sync.dma_start(out=wslab,
                                              in_=w2[dt * P:(dt + 1) * P, :])
                            for nb in range(n_bnb):
                                pt = psb.tile([P, bnb], F32, tag="ps")
                                for ft in range(NCHUNK):
                                    nc.tensor.matmul(
                                        pt[:, :], wslab[:, ts(ft, P)],
                                        g1_sb[ft][:, ts(nb, bnb)],
                                        start=(ft == 0),
                                        stop=(ft == NCHUNK - 1))
                                nc.any.tensor_copy(weff_sb[dt][:, ts(nb, bnb)],
                                                   pt[:, :])

            # --- main: outT[nt, mb] = sum_dt W_effT[dt, nt].T @ xT[dt, mb] + bias
            # (pools opened here, not alongside the build: early x-prefetch
            # DMAs jumping the queue ahead of build loads measured SLOWER on
            # HW despite modeling better)
            xs_engs = ([nc.sync, nc.scalar] if xs_q == "alt"
                       else [getattr(nc, xs_q)])
            st_engs = ([nc.scalar, nc.gpsimd] if st_q == "alt"
                       else [getattr(nc, st_q)])
            grp = mgrp * xg
            assert n_mb % grp == 0
            for _rep in range(main_reps):
                with tc.tile_pool(name="mn", bufs=xs_bufs) as mn, \
                     tc.tile_pool(name="ob", bufs=4) as ob, \
                     tc.tile_pool(name="psm", bufs=8, space="PSUM") as psm:
                    for mbg in range(n_mb // grp):

                        def emit_out(i, mb, nt, pt):
                            if wst:
                                nc.scalar.activation(
                                    osb2s[i][:, ts(nt, NB)], pt[:, :],
                                    mybir.ActivationFunctionType.Identity,
                                    bias=bias_sb[:, nt:nt + 1])
                                if nt == n_nt - 1 and not nostore:
                                    st_engs[mb % len(st_engs)].dma_start(
                                        out=outt[mb * P:(mb + 1) * P, :],
                                        in_=osb2s[i])
                                return
                            osb = ob.tile([P, NB], ODT, tag="osb")
                            nc.scalar.activation(
                                osb, pt[:, :],
                                mybir.ActivationFunctionType.Identity,
                                bias=bias_sb[:, nt:nt + 1])
                            # out-stores on a third queue, away from both
                            # the build loads (SP) and the x loads
                            idx = mb * n_nt + nt
                            if not nostore:
                                st_engs[idx % len(st_engs)].dma_start(
                                    out=outt[idx * P:(idx + 1) * P, :],
                                    in_=osb)

                        xs_list = []   # (mb, AP) per batch slab
                        if xg > 1:
                            if nox:
                                xst = pre_tiles[0]
                            elif _rep == 0 and mbg < pre_x:
                                xst = pre_tiles[mbg]
                            else:
                                xst = mn.tile([P, xg * NCHUNK * NB], IDT,
                                              tag="xs")
                                xs_engs[mbg % len(xs_engs)].dma_start(
                                    out=xst,
                                    in_=xt[mbg * P:(mbg + 1) * P, :])
                            xs_list = [
                                (mbg * xg + s,
                                 xst[:, s * NCHUNK * NB:
                                     (s + 1) * NCHUNK * NB])
                                for s in range(xg)]
                        else:
                            for i in range(mgrp):
                                mb = mbg * mgrp + i
                                if nox:
                                    xs = pre_tiles[0]
                                elif _rep == 0 and mb < pre_x:
                                    xs = pre_tiles[mb]
                                else:
                                    xs = mn.tile([P, NCHUNK * NB], IDT,
                                                 tag="xs")
                                    xs_engs[mb % len(xs_engs)].dma_start(
                                        out=xs,
                                        in_=xt[mb * P:(mb + 1) * P, :])
                                xs_list.append((mb, xs))
                        osb2s = ([ob.tile([P, n_nt * NB], ODT, tag="osb",
                                          name=f"osb{_i}")
                                  for _i in range(grp)] if wst else None)
                        if xg > 1:
                            # sequential per-slab chains out of the packed
                            # x tile (proven m1 chain structure)
                            for i, (mb, xsv) in enumerate(xs_list):
                                for nt in range(n_nt):
                                    pt = psm.tile([P, NB], F32, tag="ps")
                                    for dt in range(NCHUNK):
                                        nc.tensor.matmul(
                                            pt[:, :],
                                            weff_sb[dt][:, ts(nt, P)],
                                            xsv[:, ts(dt, NB)],
                                            start=(dt == 0),
                                            stop=(dt == NCHUNK - 1))
                                    emit_out(i, mb, nt, pt)
                            continue
                        for nt in range(n_nt):
                            pts = [psm.tile([P, NB], F32, tag="ps",
                                             name=f"pt{_i}")
                                   for _i in range(mgrp)]
                            # mb-inner so the stationary weff chunk is
                            # reused across the mgrp batch blocks
                            for dt in range(NCHUNK):
                                for i in range(mgrp):
                                    nc.tensor.matmul(
                                        pts[i][:, :],
                                        weff_sb[dt][:, ts(nt, P)],
                                        xs_list[i][1][:, ts(dt, NB)],
                                        start=(dt == 0),
                                        stop=(dt == NCHUNK - 1))
                            for i in range(mgrp):
                                emit_out(i, mbg * mgrp + i, nt, pts[i])

    nc.compile()
    return nc


_NC_CACHE = {}


def _get_nc(gb, gn):
    key = (gb, gn)
    if key not in _NC_CACHE:
        _NC_CACHE[key] = build_nc(gb, gn)
    return _NC_CACHE[key]


# ------------------------------------------------------------------ driver

def _make_in_maps(x, w_in, w_out, b_out, a_pad, b_pad, gb=GB, gn=GN,
                  io="bf16", bld2=True, xg=XG):
    np_dt = NP_BF16 if io == "bf16" else np.float32
    dstack, wopT, w2 = _host_prep(w_in, w_out, a_pad, b_pad)
    dstack = dstack.astype(np_dt)
    # bld2 streams w_in row-chunks in natural layout instead of the permuted
    # w2 (same dram tensor name/shape, different host-side contents)
    w2 = (w_in if bld2 else w2).astype(np_dt)
    x_cast = x.astype(np_dt)
    x2_by_b = {b: _per_group_x(x_cast, b, gb, xg) for b in range(gb)}
    wt_by_g = {g: _per_core_weights(b_out, wopT, g, gn, np_dt)
               for g in range(gn)}
    in_maps = []
    for core in range(N_CORES):
        b, g = divmod(core, gn)
        g0t, bias2 = wt_by_g[g]
        in_maps.append({
            "xt": x2_by_b[b],
            "w2": w2,
            "g0t": g0t,
            "dstk": dstack,
            "bias": bias2,
        })
    return in_maps


def _assemble(results, gb=GB, gn=GN):
    msh, nsl, n_mb, n_nt, _, _ = _cfg(gb, gn)
    out = np.empty((BATCH, DIM), dtype=np.float32)
    for core in range(N_CORES):
        b, g = divmod(core, gn)
        arr = results[core]["outt"].astype(np.float32)
        if arr.shape[1] == n_nt * NB:
            # wst layout: outt[mb*128+p, nt*512+m]
            arr = arr.reshape(n_mb, P, n_nt, NB)
            blk = arr.transpose(0, 3, 2, 1).reshape(msh, nsl)
        else:
            # outt rows [(mb*n_nt+nt)*128+p], cols m -> [mb, m, nt, p]
            arr = arr.reshape(n_mb, n_nt, P, NB)
            blk = arr.transpose(0, 3, 1, 2).reshape(msh, nsl)
        out[b * msh:(b + 1) * msh, g * nsl:(g + 1) * nsl] = blk
    return out


def kernel(x, w_in, w_out, b_out, a_pad, b_pad):
    x = np.asarray(x, dtype=np.float32)
    w_in = np.asarray(w_in, dtype=np.float32)
    w_out = np.asarray(w_out, dtype=np.float32)
    b_out = np.asarray(b_out, dtype=np.float32)
    a_pad = np.asarray(a_pad, dtype=np.float32)
    b_pad = np.asarray(b_pad, dtype=np.float32)
    nc = _get_nc(GB, GN)
    in_maps = _make_in_maps(x, w_in, w_out, b_out, a_pad, b_pad, GB, GN)
    res = run_bass_kernel_spmd(nc, in_maps, core_ids=list(range(N_CORES)))
    return _assemble(res.results, GB, GN)



# revision 37
# speedup vs baseline: 5.1525x; 1.0361x over previous
"""Trainium2 Bass kernel for nn_ButterflyLayer.

Reference computation:
    h   = x @ w_in.T                       [B, 2048]
    h   = butterfly(h, a_pad, b_pad)       11 stages of paired rotations
    out = h @ w_out.T + b_out              [B, 2048]

Key algebraic facts used here:
  * The butterfly is a linear map B on the 2048-dim:  out = x @ (w_out @ B @ w_in).T + b.
  * B factors as (M (x) I_128) @ blockdiag(D_0..D_15) where
      - D_c (128x128) is the composition of stages 0..6 restricted to 128-chunk c
        (those stages never mix across 128-aligned chunks), and
      - stages 7..10 use one scalar coefficient per 128-chunk, so they act as a
        16x16 matrix M on chunk indices, identically for every position inside
        a chunk.
  * So W_eff = (w_out @ (M (x) I)) @ blockdiag(D) @ w_in, and the butterfly
    costs only a block-sparse (128-wide) contraction instead of a dense one.

Host prep is limited to O(dim^2) parameter/layout work: expanding the tiny
rotation params (a_pad/b_pad) into the D_c blocks, folding the 16x16 chunk mix
M into w_out, and permuting operands into PE-friendly tiled layouts so every
device load is one large fully-contiguous DMA. All O(batch*dim^2) compute runs
on the NeuronCores.

All device data is bf16 (fp32 PSUM accumulation): rel-err ~4e-3 against the
fp32 reference (gate 2e-2), and it halves both HBM traffic and SBUF footprint
vs fp32. Same-session A/B on HW measured the bf16 kernel ~1.8x faster than
the fp32r version of the same algorithm.

Device program (SPMD on 8 cores, GB=1 batch-group x GN=8 column-groups —
every core builds a distinct 1/8 column slice of W_eff exactly once, no
duplicated build work device-wide, and streams the full batch):
  build:  g1[c]   = D_c^T-transform of the core's w_out' column slice
          W_eff^T[m, :] = sum_ft g1[ft][:, m-chunk].T @ w_in[ft-rows, :]
                  (bld2: stationary g1 m-chunks each feed 4 N=512 matmuls —
                   a fresh stationary per N=256 matmul measured ~2x slower
                   on HW because LDWEIGHTS can't hide at N=256 — then the
                   [m, J] result transposes into weff on the PE via identity
                   matmuls, tr="pe": two mc passes of 4 PSUM banks + 2
                   transpose banks. This replaced a DRAM round-trip with
                   transposing DMAs that gated the build->main handoff:
                   full-body slope 330 -> 305 us, build-only marginal
                   47.5 -> 18.2 us, same-session A/B 2026-08-10)
  main:   outT    = W_effT^T @ xT (+ bias), streamed over batch in 512-col
                    slabs; x loads on the SP HWDGE queue, out-stores on the
                    ACT HWDGE queue, first x slab prefetched on the ACT
                    queue during the build (pre_x)

Measured dead ends (same-session interleaved A/B, 2026-08-10 — see also
the session memory): mgrp stationary-reuse 2/4/8 (LDWEIGHTS already hides
at N=512; nox-isolated mgrp=4 == mgrp=1), wide 2KB-line out-stores (wst),
x loads alternated across both HWDGE rings, 2-slab packed x tiles (xg=2),
w_in build loads on the ACT ring, gb/gn=2x4 (348 vs 306 us). Main loop
runs ~236 us PE-pure vs 221 us gap-table floor; the residual is NX issue +
HAM micro-idle overhead, not LDWEIGHTS or DMA.
"""

import sys

if "/opt/trn_rl_repo" not in sys.path:
    sys.path.insert(0, "/opt/trn_rl_repo")

import numpy as np
import ml_dtypes

import concourse.bass as bass
import concourse.mybir as mybir
import concourse.tile as tile
from concourse import bacc
from concourse.bass import ts
from concourse.bass_utils import run_bass_kernel_spmd

DIM = 2048
LOG_DIM = 11
BATCH = 16384
N_CORES = 8
P = 128                # partitions
NB = 512               # main-loop moving free dim (one PSUM bank of fp32)
NCHUNK = DIM // P      # 16
F32 = mybir.dt.float32
BF16 = mybir.dt.bfloat16
NP_BF16 = ml_dtypes.bfloat16

# default sharding: GB batch-groups x GN column-groups (GB*GN == 8).
# GB=1/GN=8: every core builds a distinct 1/8 column-slice of W_eff exactly
# once (minimal PE work) and streams the full batch (DMA-heavier but still
# under the PE roofline). 2x4 re-measured 2026-08-10: full-body slope 348 vs
# 306 us — the doubled build cost dominates the halved x traffic; dead.
GB = 1
GN = 8
# packed-x group size (xg): 2 measured slightly SLOWER (312 vs 307) — keep 1
XG = 1


def _cfg(gb, gn):
    msh = BATCH // gb          # batch rows per core
    nsl = DIM // gn            # output columns per core
    n_mb = msh // NB           # 512-wide batch blocks
    n_nt = nsl // P            # 128-wide out-column tiles
    bnb = min(NB, nsl)         # build free width
    n_bnb = nsl // bnb         # build column blocks
    return msh, nsl, n_mb, n_nt, bnb, n_bnb


# ---------------------------------------------------------------- host math

def _butterfly_dense(a_pad, b_pad, stages):
    """Dense matrix of the butterfly restricted to `stages` (float64).

    Returns Bm with butterfly(v) = Bm @ v for v in R^DIM.
    """
    x = np.eye(DIM, dtype=np.float64)  # rows: basis vectors
    for l in stages:
        bs = 1 << l
        nb = DIM // (2 * bs)
        a = a_pad[l, :nb].astype(np.float64)[None, :, None]
        b = b_pad[l, :nb].astype(np.float64)[None, :, None]
        xv = x.reshape(DIM, nb, 2, bs)
        x0 = xv[:, :, 0, :]
        x1 = xv[:, :, 1, :]
        top = a * x0 + b * x1
        bot = -b * x0 + a * x1
        x = np.stack([top, bot], axis=2).reshape(DIM, DIM)
    return x.T  # butterfly(I)[r] = Bm @ e_r, so butterfly(I) = Bm.T


def _host_prep(w_in, w_out, a_pad, b_pad):
    """Expand butterfly params; permute operands into tiled device layouts."""
    d_full = _butterfly_dense(a_pad, b_pad, range(7))           # blockdiag(D_c)
    m_full = _butterfly_dense(a_pad, b_pad, range(7, LOG_DIM))  # M (x) I_128
    m_small = np.ascontiguousarray(m_full[::P, ::P])            # [16, 16]

    # dstack[k, c*128+j] = D_c[k, j]  (one contiguous [128, 2048] tile row)
    d_arr = np.stack(
        [d_full[c * P:(c + 1) * P, c * P:(c + 1) * P] for c in range(NCHUNK)]
    )                                                           # [c, k, j]
    dstack = np.ascontiguousarray(
        d_arr.transpose(1, 0, 2).reshape(P, DIM)).astype(np.float32)

    # w_out' = w_out @ (M (x) I)
    w_out64 = w_out.astype(np.float64).reshape(DIM, NCHUNK, P)
    w_out_p = np.einsum("icj,cd->idj", w_out64, m_small).reshape(DIM, DIM)
    wopT = np.ascontiguousarray(w_out_p.T).astype(np.float32)   # [k, n]

    # w2[dt*128+p, ft*128+d] = w_in[ft*128+p, dt*128+d]
    w2 = np.ascontiguousarray(
        w_in.reshape(NCHUNK, P, NCHUNK, P).transpose(2, 1, 0, 3)
        .reshape(DIM, DIM))
    return dstack, wopT, w2


def _per_core_weights(b_out, wopT, g, gn, np_dt):
    _, nsl, _, n_nt, bnb, n_bnb = _cfg(1, gn)
    # g0t[nb*128+p, c*bnb+m] = wopT[c*128+p, g*nsl + nb*bnb+m]
    ws = wopT[:, g * nsl:(g + 1) * nsl]
    g0t = np.ascontiguousarray(
        ws.reshape(NCHUNK, P, n_bnb, bnb).transpose(2, 1, 0, 3)
        .reshape(n_bnb * P, NCHUNK * bnb)).astype(np_dt)
    # bias2[p, nt] = b_out[g*nsl + nt*128 + p]
    bias2 = np.ascontiguousarray(
        b_out[g * nsl:(g + 1) * nsl].reshape(n_nt, P).T).astype(np.float32)
    return g0t, bias2


def _per_group_x(x_cast, b, gb, xg=1):
    msh, _, n_mb, _, _, _ = _cfg(gb, 1)
    # x2[mb*128+p, dt*512+m] = x[b*msh + mb*512+m, dt*128+p]
    # xg>1: x2[g*128+p, (s*16+dt)*512+m] = x[b*msh + (g*xg+s)*512+m, dt*128+p]
    xs = x_cast[b * msh:(b + 1) * msh, :]
    n_grp = n_mb // xg
    return np.ascontiguousarray(
        xs.reshape(n_grp, xg, NB, NCHUNK, P).transpose(0, 4, 1, 3, 2)
        .reshape(n_grp * P, xg * NCHUNK * NB))


# ------------------------------------------------------------- device build

def build_nc(gb=GB, gn=GN, xs_bufs=3, wslab_bufs=6, warmup=8,
             build_reps=1, main_reps=1, io="bf16", xs_q="sync",
             st_q="scalar", pre_x=2, pre_x_q="scalar", g0_whole=True,
             bld2=True, mgrp=1, tr="pe", nox=False, nostore=False,
             wst=False, wsl_q="sync", xg=XG, ws_bufs=2, hoist=True):
    msh, nsl, n_mb, n_nt, bnb, n_bnb = _cfg(gb, gn)
    IDT = BF16 if io == "bf16" else mybir.dt.float32r
    ODT = BF16 if io == "bf16" else F32
    assert not (bld2 and io != "bf16"), "bld2 needs 2-byte transposing DMA"
    assert not (bld2 and n_bnb != 1), "bld2 assumes a single build col block"
    nc = bacc.Bacc("TRN2", target_bir_lowering=False, debug=False,
                   num_devices=N_CORES)

    # xg>1 packs xg consecutive batch slabs into one DRAM row-block /
    # SBUF tile, halving the x-load DMA count and doubling the contiguous
    # line length (16KB -> 32KB per partition row)
    assert n_mb % xg == 0 and not (xg > 1 and mgrp > 1)
    xt = nc.dram_tensor("xt", [(n_mb // xg) * P, xg * NCHUNK * NB], IDT,
                        kind="ExternalInput")
    # bld2 consumes w_in row-chunks in natural layout; the classic build
    # consumes the (dt,ft)-permuted w2 layout
    w2 = nc.dram_tensor("w2", [DIM, DIM], IDT, kind="ExternalInput")
    g0t = nc.dram_tensor("g0t", [n_bnb * P, NCHUNK * bnb], IDT,
                         kind="ExternalInput")
    dstk = nc.dram_tensor("dstk", [P, DIM], IDT, kind="ExternalInput")
    bias = nc.dram_tensor("bias", [P, n_nt], F32, kind="ExternalInput")
    # outt[(mb*n_nt+nt)*128+p, m] = out[b*msh+mb*512+m, g*nsl+nt*128+p]
    # wst: outt[mb*128+p, nt*512+m] — both nt tiles of a batch slab merge
    # into one store with 2KB contiguous DRAM lines (vs 1KB), which the
    # SDMA engines move more efficiently. _assemble sniffs the layout from
    # the result shape.
    if wst:
        outt = nc.dram_tensor("outt", [n_mb * P, n_nt * NB], ODT,
                              kind="ExternalOutput")
    else:
        outt = nc.dram_tensor("outt", [n_mb * n_nt * P, NB], ODT,
                              kind="ExternalOutput")

    with tile.TileContext(nc) as tc:
        with tc.tile_pool(name="geom", bufs=1) as geom:          # persistent
            # --- persistent tiles
            dblk_sb = geom.tile([P, DIM], IDT)
            nc.sync.dma_start(out=dblk_sb, in_=dstk[:, :])
            bias_sb = geom.tile([P, n_nt], F32)
            nc.sync.dma_start(out=bias_sb, in_=bias[:, :])
            weff_sb = [geom.tile([P, nsl], IDT, name=f"weff{dt}")
                       for dt in range(NCHUNK)]
            if tr == "pe":
                from concourse.masks import make_identity
                ident = geom.tile([P, P], IDT, name="ident")
                make_identity(nc, ident)
            # PE warmup while the first DMAs land (HAM ramp)
            if warmup:
                wup = geom.tile([P, NB], BF16, name="wup")
                nc.vector.memset(wup, 0.0)
                with tc.tile_pool(name="psw", bufs=2, space="PSUM") as psw:
                    for _ in range(warmup):
                        ptw = psw.tile([P, NB], F32, tag="ps")
                        nc.tensor.matmul(ptw[:, :], wup[:, :P], wup,
                                         start=True, stop=True)

            # optional early prefetch of the first x slabs on a spare queue
            # (rep 0 of the main loop consumes these without re-loading)
            pre_tiles = []
            for i in range(pre_x):
                xsp = geom.tile([P, xg * NCHUNK * NB], IDT,
                                name=f"xs_pre{i}")
                getattr(nc, pre_x_q).dma_start(
                    out=xsp, in_=xt[i * P:(i + 1) * P, :])
                pre_tiles.append(xsp)

            # --- build W_effT = (w_in.T @ blockdiag(D).T @ w_out'.T)[:, n-slice]
            # (hoist=True opens the pools OUTSIDE the rep loop: a per-rep
            # pool open/close is a barrier, so repeated builds couldn't
            # overlap rep r+1's loads with rep r's compute and the slope
            # over-charged the build; with shared tag rings the deps are
            # per-tile instead. hoist=False reproduces the original per-rep
            # structure, kept for calibration anchoring.)
            from contextlib import ExitStack

            bstack = ExitStack()

            def _open_build_pools():
                return (bstack.enter_context(tc.tile_pool(name="bld",
                                                          bufs=2)),
                        bstack.enter_context(tc.tile_pool(name="g1p",
                                                          bufs=1)),
                        bstack.enter_context(tc.tile_pool(name="psb", bufs=8,
                                                          space="PSUM")))

            if hoist:
                bld, g1p, psb = _open_build_pools()
            if True:
                for _rep in range(build_reps):
                    if not hoist:
                        bld, g1p, psb = _open_build_pools()
                    g1_sb = [g1p.tile([P, nsl], IDT, name=f"g1_{c}",
                                      tag=f"g1_{c}", bufs=1)
                             for c in range(NCHUNK)]
                    # g1[c] = D_c^T-transform of w_out'^T chunk c
                    for nb in range(n_bnb):
                        if g0_whole:
                            g0row = bld.tile([P, NCHUNK * bnb], IDT,
                                             tag="g0w", bufs=2)
                            nc.sync.dma_start(
                                out=g0row, in_=g0t[nb * P:(nb + 1) * P, :])
                        for c in range(NCHUNK):
                            if g0_whole:
                                g0c = g0row[:, ts(c, bnb)]
                            else:
                                g0c = bld.tile([P, bnb], IDT, tag="g0",
                                               bufs=4)
                                nc.sync.dma_start(
                                    out=g0c,
                                    in_=g0t[nb * P:(nb + 1) * P, ts(c, bnb)])
                            pt = psb.tile([P, bnb], F32, tag="ps",
                                          bufs=(2 if tr == "pe" else None))
                            nc.tensor.matmul(pt[:, :], dblk_sb[:, ts(c, P)],
                                             g0c, start=True, stop=True)
                            nc.any.tensor_copy(g1_sb[c][:, ts(nb, bnb)],
                                               pt[:, :])
                    if bld2 and tr == "pe":
                        # Same math as the tr="dma" branch below, but the
                        # [m, J] -> [J, m] transpose runs on the PE (identity
                        # matmul) instead of a DRAM round-trip, removing the
                        # store+transposing-load chain that gates the
                        # build->main handoff. Two mc passes of 4 PSUM banks
                        # each leave PSUM room for the transpose outputs;
                        # w_in row slabs are pinned in SBUF across both
                        # passes so they stream only once.
                        n_mc = nsl // P
                        n_q = DIM // NB
                        wslabs = []
                        for ft in range(NCHUNK):
                            wr = bld.tile([P, DIM], IDT, tag=f"ws{ft}",
                                          bufs=ws_bufs)
                            # wsl_q="scalar" puts these on the ACT ring so
                            # rep r+1's 8 MB of w_in prefetches during rep
                            # r's main loop instead of queueing behind its
                            # x loads on the SP ring
                            getattr(nc, wsl_q).dma_start(
                                out=wr, in_=w2[ft * P:(ft + 1) * P, :])
                            wslabs.append(wr)
                        for mc in range(n_mc):
                            pts = [psb.tile([P, NB], F32, tag="mm",
                                            name=f"pt{mc}_{q}", bufs=4)
                                   for q in range(n_q)]
                            for ft in range(NCHUNK):
                                for q in range(n_q):
                                    nc.tensor.matmul(
                                        pts[q][:, :],
                                        g1_sb[ft][:, ts(mc, P)],
                                        wslabs[ft][:, ts(q, NB)],
                                        start=(ft == 0),
                                        stop=(ft == NCHUNK - 1))
                            stage = bld.tile([P, DIM], IDT, tag="stg",
                                             bufs=2)
                            for q in range(n_q):
                                nc.any.tensor_copy(stage[:, ts(q, NB)],
                                                   pts[q][:, :])
                                for dt in range(q * (NB // P),
                                                (q + 1) * (NB // P)):
                                    ptr = psb.tile([P, P], BF16, tag="tr",
                                                   bufs=2)
                                    nc.tensor.transpose(
                                        ptr[:, :], stage[:, ts(dt, P)],
                                        ident)
                                    nc.any.tensor_copy(
                                        weff_sb[dt][:, ts(mc, P)],
                                        ptr[:, :])
                    elif bld2:
                        # W_eff[m, J] = sum_ft g1[ft][:,m].T @ w_in[ft-rows, J]
                        # — stationary g1 m-chunks feed 4 N=512 matmuls each
                        # (vs a fresh stationary per N=256 matmul), then the
                        # [m, J] result is transposed back into weff_sb[dt]
                        # via a DRAM round-trip with transposing DMAs.
                        n_mc = nsl // P
                        n_q = DIM // NB
                        with tc.tile_pool(name="wt", bufs=1,
                                          space="DRAM") as wtp:
                            wtmp = wtp.tile([nsl, DIM], IDT)
                            pts = [[psb.tile([P, NB], F32, tag="ps",
                                             name=f"pt{_mc}_{_q}")
                                    for _q in range(n_q)]
                                   for _mc in range(n_mc)]
                            for ft in range(NCHUNK):
                                wr = bld.tile([P, DIM], IDT, tag="wslab",
                                              bufs=wslab_bufs)
                                nc.sync.dma_start(
                                    out=wr, in_=w2[ft * P:(ft + 1) * P, :])
                                for mc in range(n_mc):
                                    for q in range(n_q):
                                        nc.tensor.matmul(
                                            pts[mc][q][:, :],
                                            g1_sb[ft][:, ts(mc, P)],
                                            wr[:, ts(q, NB)],
                                            start=(ft == 0),
                                            stop=(ft == NCHUNK - 1))
                            for mc in range(n_mc):
                                stage = bld.tile([P, DIM], IDT, tag="stg",
                                                 bufs=2)
                                for q in range(n_q):
                                    nc.any.tensor_copy(stage[:, ts(q, NB)],
                                                       pts[mc][q][:, :])
                                nc.sync.dma_start(
                                    out=wtmp[mc * P:(mc + 1) * P, :],
                                    in_=stage)
                            # transposes stay on the SAME queue as the
                            # wtmp stage stores: queue FIFO is what orders
                            # the DRAM read-after-write (cross-queue DRAM
                            # deps are NOT tracked — splitting the chain
                            # across queues raced and corrupted weff; and
                            # per-[128,128]-chunk transposes fall below the
                            # xbar tile size and degrade to slow AP-swap
                            # descriptors, +32 us in sim)
                            for dt in range(NCHUNK):
                                nc.sync.dma_start_transpose(
                                    out=weff_sb[dt][:, :],
                                    in_=wtmp[:, ts(dt, P)])
                    else:
                        # W_effT[dt] = sum_ft w_in[ft, dt].T @ g1[ft]
                        for dt in range(NCHUNK):
                            wslab = bld.tile([P, DIM], IDT, tag="wslab",
                                             bufs=wslab_bufs)
                            nc.sync.dma_start(out=wslab,
                                              in_=w2[dt * P:(dt + 1) * P, :])
                            for nb in range(n_bnb):
                                pt = psb.tile([P, bnb], F32, tag="ps")
                                for ft in range(NCHUNK):
                                    nc.tensor.matmul(
                                        pt[:, :], wslab[:, ts(ft, P)],
                                        g1_sb[ft][:, ts(nb, bnb)],
                                        start=(ft == 0),
                                        stop=(ft == NCHUNK - 1))
                                nc.any.tensor_copy(weff_sb[dt][:, ts(nb, bnb)],
                                                   pt[:, :])
                    if not hoist:
                        bstack.close()
                        bstack = ExitStack()
                bstack.close()

            # --- main: outT[nt, mb] = sum_dt W_effT[dt, nt].T @ xT[dt, mb] + bias
            # (pools opened here, not alongside the build: early x-prefetch
            # DMAs jumping the queue ahead of build loads measured SLOWER on
            # HW despite modeling better)
            xs_engs = ([nc.sync, nc.scalar] if xs_q == "alt"
                       else [getattr(nc, xs_q)])
            st_engs = ([nc.scalar, nc.gpsimd] if st_q == "alt"
                       else [getattr(nc, st_q)])
            grp = mgrp * xg
            assert n_mb % grp == 0
            for _rep in range(main_reps):
                with tc.tile_pool(name="mn", bufs=xs_bufs) as mn, \
                     tc.tile_pool(name="ob", bufs=4) as ob, \
                     tc.tile_pool(name="psm", bufs=8, space="PSUM") as psm:
                    for mbg in range(n_mb // grp):

                        def emit_out(i, mb, nt, pt):
                            if wst:
                                nc.scalar.activation(
                                    osb2s[i][:, ts(nt, NB)], pt[:, :],
                                    mybir.ActivationFunctionType.Identity,
                                    bias=bias_sb[:, nt:nt + 1])
                                if nt == n_nt - 1 and not nostore:
                                    st_engs[mb % len(st_engs)].dma_start(
                                        out=outt[mb * P:(mb + 1) * P, :],
                                        in_=osb2s[i])
                                return
                            osb = ob.tile([P, NB], ODT, tag="osb")
                            nc.scalar.activation(
                                osb, pt[:, :],
                                mybir.ActivationFunctionType.Identity,
                                bias=bias_sb[:, nt:nt + 1])
                            # out-stores on a third queue, away from both
                            # the build loads (SP) and the x loads
                            idx = mb * n_nt + nt
                            if not nostore:
                                st_engs[idx % len(st_engs)].dma_start(
                                    out=outt[idx * P:(idx + 1) * P, :],
                                    in_=osb)

                        xs_list = []   # (mb, AP) per batch slab
                        if xg > 1:
                            if nox:
                                xst = pre_tiles[0]
                            elif _rep == 0 and mbg < pre_x:
                                xst = pre_tiles[mbg]
                            else:
                                xst = mn.tile([P, xg * NCHUNK * NB], IDT,
                                              tag="xs")
                                xs_engs[mbg % len(xs_engs)].dma_start(
                                    out=xst,
                                    in_=xt[mbg * P:(mbg + 1) * P, :])
                            xs_list = [
                                (mbg * xg + s,
                                 xst[:, s * NCHUNK * NB:
                                     (s + 1) * NCHUNK * NB])
                                for s in range(xg)]
                        else:
                            for i in range(mgrp):
                                mb = mbg * mgrp + i
                                if nox:
                                    xs = pre_tiles[0]
                                elif _rep == 0 and mb < pre_x:
                                    xs = pre_tiles[mb]
                                else:
                                    xs = mn.tile([P, NCHUNK * NB], IDT,
                                                 tag="xs")
                                    xs_engs[mb % len(xs_engs)].dma_start(
                                        out=xs,
                                        in_=xt[mb * P:(mb + 1) * P, :])
                                xs_list.append((mb, xs))
                        osb2s = ([ob.tile([P, n_nt * NB], ODT, tag="osb",
                                          name=f"osb{_i}")
                                  for _i in range(grp)] if wst else None)
                        if xg > 1:
                            # sequential per-slab chains out of the packed
                            # x tile (proven m1 chain structure)
                            for i, (mb, xsv) in enumerate(xs_list):
                                for nt in range(n_nt):
                                    pt = psm.tile([P, NB], F32, tag="ps")
                                    for dt in range(NCHUNK):
                                        nc.tensor.matmul(
                                            pt[:, :],
                                            weff_sb[dt][:, ts(nt, P)],
                                            xsv[:, ts(dt, NB)],
                                            start=(dt == 0),
                                            stop=(dt == NCHUNK - 1))
                                    emit_out(i, mb, nt, pt)
                            continue
                        for nt in range(n_nt):
                            pts = [psm.tile([P, NB], F32, tag="ps",
                                             name=f"pt{_i}")
                                   for _i in range(mgrp)]
                            # mb-inner so the stationary weff chunk is
                            # reused across the mgrp batch blocks
                            for dt in range(NCHUNK):
                                for i in range(mgrp):
                                    nc.tensor.matmul(
                                        pts[i][:, :],
                                        weff_sb[dt][:, ts(nt, P)],
                                        xs_list[i][1][:, ts(dt, NB)],
                                        start=(dt == 0),
                                        stop=(dt == NCHUNK - 1))
                            for i in range(mgrp):
                                emit_out(i, mbg * mgrp + i, nt, pts[i])

    nc.compile()
    return nc


_NC_CACHE = {}


def _get_nc(gb, gn):
    key = (gb, gn)
    if key not in _NC_CACHE:
        _NC_CACHE[key] = build_nc(gb, gn)
    return _NC_CACHE[key]


# ------------------------------------------------------------------ driver

def _make_in_maps(x, w_in, w_out, b_out, a_pad, b_pad, gb=GB, gn=GN,
                  io="bf16", bld2=True, xg=XG):
    np_dt = NP_BF16 if io == "bf16" else np.float32
    dstack, wopT, w2 = _host_prep(w_in, w_out, a_pad, b_pad)
    dstack = dstack.astype(np_dt)
    # bld2 streams w_in row-chunks in natural layout instead of the permuted
    # w2 (same dram tensor name/shape, different host-side contents)
    w2 = (w_in if bld2 else w2).astype(np_dt)
    x_cast = x.astype(np_dt)
    x2_by_b = {b: _per_group_x(x_cast, b, gb, xg) for b in range(gb)}
    wt_by_g = {g: _per_core_weights(b_out, wopT, g, gn, np_dt)
               for g in range(gn)}
    in_maps = []
    for core in range(N_CORES):
        b, g = divmod(core, gn)
        g0t, bias2 = wt_by_g[g]
        in_maps.append({
            "xt": x2_by_b[b],
            "w2": w2,
            "g0t": g0t,
            "dstk": dstack,
            "bias": bias2,
        })
    return in_maps


def _assemble(results, gb=GB, gn=GN):
    msh, nsl, n_mb, n_nt, _, _ = _cfg(gb, gn)
    out = np.empty((BATCH, DIM), dtype=np.float32)
    for core in range(N_CORES):
        b, g = divmod(core, gn)
        arr = results[core]["outt"].astype(np.float32)
        if arr.shape[1] == n_nt * NB:
            # wst layout: outt[mb*128+p, nt*512+m]
            arr = arr.reshape(n_mb, P, n_nt, NB)
            blk = arr.transpose(0, 3, 2, 1).reshape(msh, nsl)
        else:
            # outt rows [(mb*n_nt+nt)*128+p], cols m -> [mb, m, nt, p]
            arr = arr.reshape(n_mb, n_nt, P, NB)
            blk = arr.transpose(0, 3, 1, 2).reshape(msh, nsl)
        out[b * msh:(b + 1) * msh, g * nsl:(g + 1) * nsl] = blk
    return out


def kernel(x, w_in, w_out, b_out, a_pad, b_pad):
    x = np.asarray(x, dtype=np.float32)
    w_in = np.asarray(w_in, dtype=np.float32)
    w_out = np.asarray(w_out, dtype=np.float32)
    b_out = np.asarray(b_out, dtype=np.float32)
    a_pad = np.asarray(a_pad, dtype=np.float32)
    b_pad = np.asarray(b_pad, dtype=np.float32)
    nc = _get_nc(GB, GN)
    in_maps = _make_in_maps(x, w_in, w_out, b_out, a_pad, b_pad, GB, GN)
    res = run_bass_kernel_spmd(nc, in_maps, core_ids=list(range(N_CORES)))
    return _assemble(res.results, GB, GN)



# revision 39
# speedup vs baseline: 5.3016x; 1.0289x over previous
"""Trainium2 Bass kernel for nn_ButterflyLayer.

Reference computation:
    h   = x @ w_in.T                       [B, 2048]
    h   = butterfly(h, a_pad, b_pad)       11 stages of paired rotations
    out = h @ w_out.T + b_out              [B, 2048]

Key algebraic facts used here:
  * The butterfly is a linear map B on the 2048-dim:  out = x @ (w_out @ B @ w_in).T + b.
  * B factors as (M (x) I_128) @ blockdiag(D_0..D_15) where
      - D_c (128x128) is the composition of stages 0..6 restricted to 128-chunk c
        (those stages never mix across 128-aligned chunks), and
      - stages 7..10 use one scalar coefficient per 128-chunk, so they act as a
        16x16 matrix M on chunk indices, identically for every position inside
        a chunk.
  * So W_eff = (w_out @ (M (x) I)) @ blockdiag(D) @ w_in, and the butterfly
    costs only a block-sparse (128-wide) contraction instead of a dense one.

Host prep is limited to O(dim^2) parameter/layout work: expanding the tiny
rotation params (a_pad/b_pad) into the D_c blocks, folding the 16x16 chunk mix
M into w_out, and permuting operands into PE-friendly tiled layouts so every
device load is one large fully-contiguous DMA. All O(batch*dim^2) compute runs
on the NeuronCores.

All device data is bf16 (fp32 PSUM accumulation): rel-err ~4e-3 against the
fp32 reference (gate 2e-2), and it halves both HBM traffic and SBUF footprint
vs fp32. Same-session A/B on HW measured the bf16 kernel ~1.8x faster than
the fp32r version of the same algorithm.

Device program (SPMD on 8 cores, GB=1 batch-group x GN=8 column-groups —
every core builds a distinct 1/8 column slice of W_eff exactly once, no
duplicated build work device-wide, and streams the full batch):
  build:  g1[c]   = D_c^T-transform of the core's w_out' column slice
          W_eff^T[m, :] = sum_ft g1[ft][:, m-chunk].T @ w_in[ft-rows, :]
                  (bld2: stationary g1 m-chunks each feed 4 N=512 matmuls —
                   a fresh stationary per N=256 matmul measured ~2x slower
                   on HW because LDWEIGHTS can't hide at N=256 — then the
                   [m, J] result transposes into weff on the PE via identity
                   matmuls, tr="pe": two mc passes of 4 PSUM banks + 2
                   transpose banks. This replaced a DRAM round-trip with
                   transposing DMAs that gated the build->main handoff:
                   full-body slope 330 -> 305 us, build-only marginal
                   47.5 -> 18.2 us, same-session A/B 2026-08-10)
  main:   outT    = W_effT^T @ xT (+ bias), streamed over batch in 512-col
                    slabs; x loads on the SP HWDGE queue, out-stores on the
                    ACT HWDGE queue, first x slab prefetched on the ACT
                    queue during the build (pre_x)

Measured dead ends (same-session interleaved A/B, 2026-08-10 — see also
the session memory): mgrp stationary-reuse 2/4/8 (LDWEIGHTS already hides
at N=512; nox-isolated mgrp=4 == mgrp=1), wide 2KB-line out-stores (wst),
x loads alternated across both HWDGE rings, 2-slab packed x tiles (xg=2),
w_in build loads on the ACT ring, gb/gn=2x4 (348 vs 306 us). Main loop
runs ~236 us PE-pure vs 221 us gap-table floor; the residual is NX issue +
HAM micro-idle overhead, not LDWEIGHTS or DMA.
"""

import sys

if "/opt/trn_rl_repo" not in sys.path:
    sys.path.insert(0, "/opt/trn_rl_repo")

import numpy as np
import ml_dtypes

import concourse.bass as bass
import concourse.mybir as mybir
import concourse.tile as tile
from concourse import bacc
from concourse.bass import ts
from concourse.bass_utils import run_bass_kernel_spmd

DIM = 2048
LOG_DIM = 11
BATCH = 16384
N_CORES = 8
P = 128                # partitions
NB = 512               # main-loop moving free dim (one PSUM bank of fp32)
NCHUNK = DIM // P      # 16
F32 = mybir.dt.float32
BF16 = mybir.dt.bfloat16
NP_BF16 = ml_dtypes.bfloat16

# default sharding: GB batch-groups x GN column-groups (GB*GN == 8).
# GB=1/GN=8: every core builds a distinct 1/8 column-slice of W_eff exactly
# once (minimal PE work) and streams the full batch (DMA-heavier but still
# under the PE roofline). 2x4 re-measured 2026-08-10: full-body slope 348 vs
# 306 us — the doubled build cost dominates the halved x traffic; dead.
GB = 1
GN = 8
# packed-x group size (xg): 2 measured slightly SLOWER (312 vs 307) — keep 1
XG = 1


def _cfg(gb, gn):
    msh = BATCH // gb          # batch rows per core
    nsl = DIM // gn            # output columns per core
    n_mb = msh // NB           # 512-wide batch blocks
    n_nt = nsl // P            # 128-wide out-column tiles
    bnb = min(NB, nsl)         # build free width
    n_bnb = nsl // bnb         # build column blocks
    return msh, nsl, n_mb, n_nt, bnb, n_bnb


# ---------------------------------------------------------------- host math

def _butterfly_dense(a_pad, b_pad, stages):
    """Dense matrix of the butterfly restricted to `stages` (float64).

    Returns Bm with butterfly(v) = Bm @ v for v in R^DIM.
    """
    x = np.eye(DIM, dtype=np.float64)  # rows: basis vectors
    for l in stages:
        bs = 1 << l
        nb = DIM // (2 * bs)
        a = a_pad[l, :nb].astype(np.float64)[None, :, None]
        b = b_pad[l, :nb].astype(np.float64)[None, :, None]
        xv = x.reshape(DIM, nb, 2, bs)
        x0 = xv[:, :, 0, :]
        x1 = xv[:, :, 1, :]
        top = a * x0 + b * x1
        bot = -b * x0 + a * x1
        x = np.stack([top, bot], axis=2).reshape(DIM, DIM)
    return x.T  # butterfly(I)[r] = Bm @ e_r, so butterfly(I) = Bm.T


def _host_prep(w_in, w_out, a_pad, b_pad):
    """Expand butterfly params; permute operands into tiled device layouts."""
    d_full = _butterfly_dense(a_pad, b_pad, range(7))           # blockdiag(D_c)
    m_full = _butterfly_dense(a_pad, b_pad, range(7, LOG_DIM))  # M (x) I_128
    m_small = np.ascontiguousarray(m_full[::P, ::P])            # [16, 16]

    # dstack[k, c*128+j] = D_c[k, j]  (one contiguous [128, 2048] tile row)
    d_arr = np.stack(
        [d_full[c * P:(c + 1) * P, c * P:(c + 1) * P] for c in range(NCHUNK)]
    )                                                           # [c, k, j]
    dstack = np.ascontiguousarray(
        d_arr.transpose(1, 0, 2).reshape(P, DIM)).astype(np.float32)

    # w_out' = w_out @ (M (x) I)
    w_out64 = w_out.astype(np.float64).reshape(DIM, NCHUNK, P)
    w_out_p = np.einsum("icj,cd->idj", w_out64, m_small).reshape(DIM, DIM)
    wopT = np.ascontiguousarray(w_out_p.T).astype(np.float32)   # [k, n]

    # w2[dt*128+p, ft*128+d] = w_in[ft*128+p, dt*128+d]
    w2 = np.ascontiguousarray(
        w_in.reshape(NCHUNK, P, NCHUNK, P).transpose(2, 1, 0, 3)
        .reshape(DIM, DIM))
    return dstack, wopT, w2


def _per_core_weights(b_out, wopT, g, gn, np_dt):
    _, nsl, _, n_nt, bnb, n_bnb = _cfg(1, gn)
    # g0t[nb*128+p, c*bnb+m] = wopT[c*128+p, g*nsl + nb*bnb+m]
    ws = wopT[:, g * nsl:(g + 1) * nsl]
    g0t = np.ascontiguousarray(
        ws.reshape(NCHUNK, P, n_bnb, bnb).transpose(2, 1, 0, 3)
        .reshape(n_bnb * P, NCHUNK * bnb)).astype(np_dt)
    # bias2[p, nt] = b_out[g*nsl + nt*128 + p]
    bias2 = np.ascontiguousarray(
        b_out[g * nsl:(g + 1) * nsl].reshape(n_nt, P).T).astype(np.float32)
    return g0t, bias2


def _per_group_x(x_cast, b, gb, xg=1):
    msh, _, n_mb, _, _, _ = _cfg(gb, 1)
    # x2[mb*128+p, dt*512+m] = x[b*msh + mb*512+m, dt*128+p]
    # xg>1: x2[g*128+p, (s*16+dt)*512+m] = x[b*msh + (g*xg+s)*512+m, dt*128+p]
    xs = x_cast[b * msh:(b + 1) * msh, :]
    n_grp = n_mb // xg
    return np.ascontiguousarray(
        xs.reshape(n_grp, xg, NB, NCHUNK, P).transpose(0, 4, 1, 3, 2)
        .reshape(n_grp * P, xg * NCHUNK * NB))


# ------------------------------------------------------------- device build

def build_nc(gb=GB, gn=GN, xs_bufs=3, wslab_bufs=6, warmup=8,
             build_reps=1, main_reps=1, io="bf16", xs_q="sync",
             st_q="scalar", pre_x=2, pre_x_q="scalar", g0_whole=True,
             bld2=True, mgrp=1, tr="pe", nox=False, nostore=False,
             wst=False, wsl_q="sync", xg=XG, ws_bufs=2, hoist=True):
    msh, nsl, n_mb, n_nt, bnb, n_bnb = _cfg(gb, gn)
    IDT = BF16 if io == "bf16" else mybir.dt.float32r
    ODT = BF16 if io == "bf16" else F32
    assert not (bld2 and io != "bf16"), "bld2 needs 2-byte transposing DMA"
    assert not (bld2 and n_bnb != 1), "bld2 assumes a single build col block"
    nc = bacc.Bacc("TRN2", target_bir_lowering=False, debug=False,
                   num_devices=N_CORES)

    # xg>1 packs xg consecutive batch slabs into one DRAM row-block /
    # SBUF tile, halving the x-load DMA count and doubling the contiguous
    # line length (16KB -> 32KB per partition row)
    assert n_mb % xg == 0 and not (xg > 1 and mgrp > 1)
    xt = nc.dram_tensor("xt", [(n_mb // xg) * P, xg * NCHUNK * NB], IDT,
                        kind="ExternalInput")
    # bld2 consumes w_in row-chunks in natural layout; the classic build
    # consumes the (dt,ft)-permuted w2 layout
    w2 = nc.dram_tensor("w2", [DIM, DIM], IDT, kind="ExternalInput")
    g0t = nc.dram_tensor("g0t", [n_bnb * P, NCHUNK * bnb], IDT,
                         kind="ExternalInput")
    dstk = nc.dram_tensor("dstk", [P, DIM], IDT, kind="ExternalInput")
    bias = nc.dram_tensor("bias", [P, n_nt], F32, kind="ExternalInput")
    # outt[(mb*n_nt+nt)*128+p, m] = out[b*msh+mb*512+m, g*nsl+nt*128+p]
    # wst: outt[mb*128+p, nt*512+m] — both nt tiles of a batch slab merge
    # into one store with 2KB contiguous DRAM lines (vs 1KB), which the
    # SDMA engines move more efficiently. _assemble sniffs the layout from
    # the result shape.
    if wst:
        outt = nc.dram_tensor("outt", [n_mb * P, n_nt * NB], ODT,
                              kind="ExternalOutput")
    else:
        outt = nc.dram_tensor("outt", [n_mb * n_nt * P, NB], ODT,
                              kind="ExternalOutput")

    with tile.TileContext(nc) as tc:
        with tc.tile_pool(name="geom", bufs=1) as geom:          # persistent
            # --- persistent tiles
            dblk_sb = geom.tile([P, DIM], IDT)
            nc.sync.dma_start(out=dblk_sb, in_=dstk[:, :])
            bias_sb = geom.tile([P, n_nt], F32)
            nc.sync.dma_start(out=bias_sb, in_=bias[:, :])
            weff_sb = [geom.tile([P, nsl], IDT, name=f"weff{dt}")
                       for dt in range(NCHUNK)]
            if tr == "pe":
                from concourse.masks import make_identity
                ident = geom.tile([P, P], IDT, name="ident")
                make_identity(nc, ident)
            # PE warmup while the first DMAs land (HAM ramp)
            if warmup:
                wup = geom.tile([P, NB], BF16, name="wup")
                nc.vector.memset(wup, 0.0)
                with tc.tile_pool(name="psw", bufs=2, space="PSUM") as psw:
                    for _ in range(warmup):
                        ptw = psw.tile([P, NB], F32, tag="ps")
                        nc.tensor.matmul(ptw[:, :], wup[:, :P], wup,
                                         start=True, stop=True)

            # optional early prefetch of the first x slabs on a spare queue
            # (rep 0 of the main loop consumes these without re-loading)
            pre_tiles = []
            for i in range(pre_x):
                xsp = geom.tile([P, xg * NCHUNK * NB], IDT,
                                name=f"xs_pre{i}")
                getattr(nc, pre_x_q).dma_start(
                    out=xsp, in_=xt[i * P:(i + 1) * P, :])
                pre_tiles.append(xsp)

            # --- build W_effT = (w_in.T @ blockdiag(D).T @ w_out'.T)[:, n-slice]
            # (hoist=True opens the pools OUTSIDE the rep loop: a per-rep
            # pool open/close is a barrier, so repeated builds couldn't
            # overlap rep r+1's loads with rep r's compute and the slope
            # over-charged the build; with shared tag rings the deps are
            # per-tile instead. hoist=False reproduces the original per-rep
            # structure, kept for calibration anchoring.)
            from contextlib import ExitStack

            bstack = ExitStack()

            def _open_build_pools():
                return (bstack.enter_context(tc.tile_pool(name="bld",
                                                          bufs=2)),
                        bstack.enter_context(tc.tile_pool(name="g1p",
                                                          bufs=1)),
                        bstack.enter_context(tc.tile_pool(name="psb", bufs=8,
                                                          space="PSUM")))

            if hoist:
                bld, g1p, psb = _open_build_pools()
            if True:
                for _rep in range(build_reps):
                    if not hoist:
                        bld, g1p, psb = _open_build_pools()
                    g1_sb = [g1p.tile([P, nsl], IDT, name=f"g1_{c}",
                                      tag=f"g1_{c}", bufs=1)
                             for c in range(NCHUNK)]
                    # g1[c] = D_c^T-transform of w_out'^T chunk c
                    for nb in range(n_bnb):
                        if g0_whole:
                            g0row = bld.tile([P, NCHUNK * bnb], IDT,
                                             tag="g0w", bufs=2)
                            nc.sync.dma_start(
                                out=g0row, in_=g0t[nb * P:(nb + 1) * P, :])
                        for c in range(NCHUNK):
                            if g0_whole:
                                g0c = g0row[:, ts(c, bnb)]
                            else:
                                g0c = bld.tile([P, bnb], IDT, tag="g0",
                                               bufs=4)
                                nc.sync.dma_start(
                                    out=g0c,
                                    in_=g0t[nb * P:(nb + 1) * P, ts(c, bnb)])
                            pt = psb.tile([P, bnb], F32, tag="ps",
                                          bufs=(2 if tr == "pe" else None))
                            nc.tensor.matmul(pt[:, :], dblk_sb[:, ts(c, P)],
                                             g0c, start=True, stop=True)
                            nc.any.tensor_copy(g1_sb[c][:, ts(nb, bnb)],
                                               pt[:, :])
                    if bld2 and tr == "pe":
                        # Same math as the tr="dma" branch below, but the
                        # [m, J] -> [J, m] transpose runs on the PE (identity
                        # matmul) instead of a DRAM round-trip, removing the
                        # store+transposing-load chain that gates the
                        # build->main handoff. Two mc passes of 4 PSUM banks
                        # each leave PSUM room for the transpose outputs;
                        # w_in row slabs are pinned in SBUF across both
                        # passes so they stream only once.
                        n_mc = nsl // P
                        n_q = DIM // NB
                        wslabs = []
                        for ft in range(NCHUNK):
                            wr = bld.tile([P, DIM], IDT, tag=f"ws{ft}",
                                          bufs=ws_bufs)
                            # wsl_q="scalar" puts these on the ACT ring so
                            # rep r+1's 8 MB of w_in prefetches during rep
                            # r's main loop instead of queueing behind its
                            # x loads on the SP ring
                            getattr(nc, wsl_q).dma_start(
                                out=wr, in_=w2[ft * P:(ft + 1) * P, :])
                            wslabs.append(wr)
                        for mc in range(n_mc):
                            pts = [psb.tile([P, NB], F32, tag="mm",
                                            name=f"pt{mc}_{q}", bufs=4)
                                   for q in range(n_q)]
                            for ft in range(NCHUNK):
                                for q in range(n_q):
                                    nc.tensor.matmul(
                                        pts[q][:, :],
                                        g1_sb[ft][:, ts(mc, P)],
                                        wslabs[ft][:, ts(q, NB)],
                                        start=(ft == 0),
                                        stop=(ft == NCHUNK - 1))
                            stage = bld.tile([P, DIM], IDT, tag="stg",
                                             bufs=2)
                            for q in range(n_q):
                                nc.any.tensor_copy(stage[:, ts(q, NB)],
                                                   pts[q][:, :])
                                for dt in range(q * (NB // P),
                                                (q + 1) * (NB // P)):
                                    ptr = psb.tile([P, P], BF16, tag="tr",
                                                   bufs=2)
                                    nc.tensor.transpose(
                                        ptr[:, :], stage[:, ts(dt, P)],
                                        ident)
                                    nc.any.tensor_copy(
                                        weff_sb[dt][:, ts(mc, P)],
                                        ptr[:, :])
                    elif bld2:
                        # W_eff[m, J] = sum_ft g1[ft][:,m].T @ w_in[ft-rows, J]
                        # — stationary g1 m-chunks feed 4 N=512 matmuls each
                        # (vs a fresh stationary per N=256 matmul), then the
                        # [m, J] result is transposed back into weff_sb[dt]
                        # via a DRAM round-trip with transposing DMAs.
                        n_mc = nsl // P
                        n_q = DIM // NB
                        with tc.tile_pool(name="wt", bufs=1,
                                          space="DRAM") as wtp:
                            wtmp = wtp.tile([nsl, DIM], IDT)
                            pts = [[psb.tile([P, NB], F32, tag="ps",
                                             name=f"pt{_mc}_{_q}")
                                    for _q in range(n_q)]
                                   for _mc in range(n_mc)]
                            for ft in range(NCHUNK):
                                wr = bld.tile([P, DIM], IDT, tag="wslab",
                                              bufs=wslab_bufs)
                                nc.sync.dma_start(
                                    out=wr, in_=w2[ft * P:(ft + 1) * P, :])
                                for mc in range(n_mc):
                                    for q in range(n_q):
                                        nc.tensor.matmul(
                                            pts[mc][q][:, :],
                                            g1_sb[ft][:, ts(mc, P)],
                                            wr[:, ts(q, NB)],
                                            start=(ft == 0),
                                            stop=(ft == NCHUNK - 1))
                            for mc in range(n_mc):
                                stage = bld.tile([P, DIM], IDT, tag="stg",
                                                 bufs=2)
                                for q in range(n_q):
                                    nc.any.tensor_copy(stage[:, ts(q, NB)],
                                                       pts[mc][q][:, :])
                                nc.sync.dma_start(
                                    out=wtmp[mc * P:(mc + 1) * P, :],
                                    in_=stage)
                            # transposes stay on the SAME queue as the
                            # wtmp stage stores: queue FIFO is what orders
                            # the DRAM read-after-write (cross-queue DRAM
                            # deps are NOT tracked — splitting the chain
                            # across queues raced and corrupted weff; and
                            # per-[128,128]-chunk transposes fall below the
                            # xbar tile size and degrade to slow AP-swap
                            # descriptors, +32 us in sim)
                            for dt in range(NCHUNK):
                                nc.sync.dma_start_transpose(
                                    out=weff_sb[dt][:, :],
                                    in_=wtmp[:, ts(dt, P)])
                    else:
                        # W_effT[dt] = sum_ft w_in[ft, dt].T @ g1[ft]
                        for dt in range(NCHUNK):
                            wslab = bld.tile([P, DIM], IDT, tag="wslab",
                                             bufs=wslab_bufs)
                            nc.sync.dma_start(out=wslab,
                                              in_=w2[dt * P:(dt + 1) * P, :])
                            for nb in range(n_bnb):
                                pt = psb.tile([P, bnb], F32, tag="ps")
                                for ft in range(NCHUNK):
                                    nc.tensor.matmul(
                                        pt[:, :], wslab[:, ts(ft, P)],
                                        g1_sb[ft][:, ts(nb, bnb)],
                                        start=(ft == 0),
                                        stop=(ft == NCHUNK - 1))
                                nc.any.tensor_copy(weff_sb[dt][:, ts(nb, bnb)],
                                                   pt[:, :])
                    if not hoist:
                        bstack.close()
                        bstack = ExitStack()
                bstack.close()

            # --- main: outT[nt, mb] = sum_dt W_effT[dt, nt].T @ xT[dt, mb] + bias
            # (pools opened here, not alongside the build: early x-prefetch
            # DMAs jumping the queue ahead of build loads measured SLOWER on
            # HW despite modeling better)
            xs_engs = ([nc.sync, nc.scalar] if xs_q == "alt"
                       else [getattr(nc, xs_q)])
            st_engs = ([nc.scalar, nc.gpsimd] if st_q == "alt"
                       else [getattr(nc, st_q)])
            grp = mgrp * xg
            assert n_mb % grp == 0
            # same barrier story as the build: per-rep pool open/close
            # drains the x pipeline at every rep boundary; hoist=True keeps
            # one pool across reps so slab loads flow through boundaries
            mstack = ExitStack()

            def _open_main_pools():
                return (mstack.enter_context(
                            tc.tile_pool(name="mn", bufs=xs_bufs)),
                        mstack.enter_context(tc.tile_pool(name="ob",
                                                          bufs=4)),
                        mstack.enter_context(tc.tile_pool(name="psm", bufs=8,
                                                          space="PSUM")))

            if hoist:
                mn, ob, psm = _open_main_pools()
            if True:
                for _rep in range(main_reps):
                    if not hoist:
                        mn, ob, psm = _open_main_pools()
                    for mbg in range(n_mb // grp):

                        def emit_out(i, mb, nt, pt):
                            if wst:
                                nc.scalar.activation(
                                    osb2s[i][:, ts(nt, NB)], pt[:, :],
                                    mybir.ActivationFunctionType.Identity,
                                    bias=bias_sb[:, nt:nt + 1])
                                if nt == n_nt - 1 and not nostore:
                                    st_engs[mb % len(st_engs)].dma_start(
                                        out=outt[mb * P:(mb + 1) * P, :],
                                        in_=osb2s[i])
                                return
                            osb = ob.tile([P, NB], ODT, tag="osb")
                            nc.scalar.activation(
                                osb, pt[:, :],
                                mybir.ActivationFunctionType.Identity,
                                bias=bias_sb[:, nt:nt + 1])
                            # out-stores on a third queue, away from both
                            # the build loads (SP) and the x loads
                            idx = mb * n_nt + nt
                            if not nostore:
                                st_engs[idx % len(st_engs)].dma_start(
                                    out=outt[idx * P:(idx + 1) * P, :],
                                    in_=osb)

                        xs_list = []   # (mb, AP) per batch slab
                        if xg > 1:
                            if nox:
                                xst = pre_tiles[0]
                            elif _rep == 0 and mbg < pre_x:
                                xst = pre_tiles[mbg]
                            else:
                                xst = mn.tile([P, xg * NCHUNK * NB], IDT,
                                              tag="xs")
                                xs_engs[mbg % len(xs_engs)].dma_start(
                                    out=xst,
                                    in_=xt[mbg * P:(mbg + 1) * P, :])
                            xs_list = [
                                (mbg * xg + s,
                                 xst[:, s * NCHUNK * NB:
                                     (s + 1) * NCHUNK * NB])
                                for s in range(xg)]
                        else:
                            for i in range(mgrp):
                                mb = mbg * mgrp + i
                                if nox:
                                    xs = pre_tiles[0]
                                elif _rep == 0 and mb < pre_x:
                                    xs = pre_tiles[mb]
                                else:
                                    xs = mn.tile([P, NCHUNK * NB], IDT,
                                                 tag="xs")
                                    xs_engs[mb % len(xs_engs)].dma_start(
                                        out=xs,
                                        in_=xt[mb * P:(mb + 1) * P, :])
                                xs_list.append((mb, xs))
                        osb2s = ([ob.tile([P, n_nt * NB], ODT, tag="osb",
                                          name=f"osb{_i}")
                                  for _i in range(grp)] if wst else None)
                        if xg > 1:
                            # sequential per-slab chains out of the packed
                            # x tile (proven m1 chain structure)
                            for i, (mb, xsv) in enumerate(xs_list):
                                for nt in range(n_nt):
                                    pt = psm.tile([P, NB], F32, tag="ps")
                                    for dt in range(NCHUNK):
                                        nc.tensor.matmul(
                                            pt[:, :],
                                            weff_sb[dt][:, ts(nt, P)],
                                            xsv[:, ts(dt, NB)],
                                            start=(dt == 0),
                                            stop=(dt == NCHUNK - 1))
                                    emit_out(i, mb, nt, pt)
                            continue
                        for nt in range(n_nt):
                            pts = [psm.tile([P, NB], F32, tag="ps",
                                             name=f"pt{_i}")
                                   for _i in range(mgrp)]
                            # mb-inner so the stationary weff chunk is
                            # reused across the mgrp batch blocks
                            for dt in range(NCHUNK):
                                for i in range(mgrp):
                                    nc.tensor.matmul(
                                        pts[i][:, :],
                                        weff_sb[dt][:, ts(nt, P)],
                                        xs_list[i][1][:, ts(dt, NB)],
                                        start=(dt == 0),
                                        stop=(dt == NCHUNK - 1))
                            for i in range(mgrp):
                                emit_out(i, mbg * mgrp + i, nt, pts[i])
                    if not hoist:
                        mstack.close()
                        mstack = ExitStack()
                mstack.close()

    nc.compile()
    return nc


_NC_CACHE = {}


def _get_nc(gb, gn):
    key = (gb, gn)
    if key not in _NC_CACHE:
        _NC_CACHE[key] = build_nc(gb, gn)
    return _NC_CACHE[key]


# ------------------------------------------------------------------ driver

def _make_in_maps(x, w_in, w_out, b_out, a_pad, b_pad, gb=GB, gn=GN,
                  io="bf16", bld2=True, xg=XG):
    np_dt = NP_BF16 if io == "bf16" else np.float32
    dstack, wopT, w2 = _host_prep(w_in, w_out, a_pad, b_pad)
    dstack = dstack.astype(np_dt)
    # bld2 streams w_in row-chunks in natural layout instead of the permuted
    # w2 (same dram tensor name/shape, different host-side contents)
    w2 = (w_in if bld2 else w2).astype(np_dt)
    x_cast = x.astype(np_dt)
    x2_by_b = {b: _per_group_x(x_cast, b, gb, xg) for b in range(gb)}
    wt_by_g = {g: _per_core_weights(b_out, wopT, g, gn, np_dt)
               for g in range(gn)}
    in_maps = []
    for core in range(N_CORES):
        b, g = divmod(core, gn)
        g0t, bias2 = wt_by_g[g]
        in_maps.append({
            "xt": x2_by_b[b],
            "w2": w2,
            "g0t": g0t,
            "dstk": dstack,
            "bias": bias2,
        })
    return in_maps


def _assemble(results, gb=GB, gn=GN):
    msh, nsl, n_mb, n_nt, _, _ = _cfg(gb, gn)
    out = np.empty((BATCH, DIM), dtype=np.float32)
    for core in range(N_CORES):
        b, g = divmod(core, gn)
        arr = results[core]["outt"].astype(np.float32)
        if arr.shape[1] == n_nt * NB:
            # wst layout: outt[mb*128+p, nt*512+m]
            arr = arr.reshape(n_mb, P, n_nt, NB)
            blk = arr.transpose(0, 3, 2, 1).reshape(msh, nsl)
        else:
            # outt rows [(mb*n_nt+nt)*128+p], cols m -> [mb, m, nt, p]
            arr = arr.reshape(n_mb, n_nt, P, NB)
            blk = arr.transpose(0, 3, 1, 2).reshape(msh, nsl)
        out[b * msh:(b + 1) * msh, g * nsl:(g + 1) * nsl] = blk
    return out


def kernel(x, w_in, w_out, b_out, a_pad, b_pad):
    x = np.asarray(x, dtype=np.float32)
    w_in = np.asarray(w_in, dtype=np.float32)
    w_out = np.asarray(w_out, dtype=np.float32)
    b_out = np.asarray(b_out, dtype=np.float32)
    a_pad = np.asarray(a_pad, dtype=np.float32)
    b_pad = np.asarray(b_pad, dtype=np.float32)
    nc = _get_nc(GB, GN)
    in_maps = _make_in_maps(x, w_in, w_out, b_out, a_pad, b_pad, GB, GN)
    res = run_bass_kernel_spmd(nc, in_maps, core_ids=list(range(N_CORES)))
    return _assemble(res.results, GB, GN)

